# revision 1
# baseline (speedup 1.0000x reference)
"""GCN feature extractor on 8 Trainium2 NeuronCores.

Distribution: nodes are block-sharded over 8 cores (12500 each, padded to
12544 = 98*128). Per GCN layer each core computes its slice of dinv*(h@W),
an AllGather replicates the full [100352,64] table to DRAM, and neighbor
features are fetched with batched dma_gather instructions (int16 indices,
one instruction per destination-tile-group x 25088-row table window; pad
slots point at a zero row inside the window). Messages land slot-aligned so
one strided 4D reduce per (group, window) computes the neighbor sums.
BatchNorm statistics are combined with a tiny AllReduce. Mean+max graph
pooling gathers the final table per-column; each core pools 32 graphs and
the [256,64] result is assembled with a final AllGather.
"""

import numpy as np

N = 100000
E = 1600000
D = 64
G = 256
NC = 8
NPC_RAW = 12500
NPC = 12544          # 98 tiles of 128
NT = NPC // 128      # 98
TR = NC * NPC        # 100352 table rows
EPS = 1e-5
ZPAD = NPC_RAW       # local row 12500 of any core block is always (near) zero
GPC = G // NC        # 32 graphs per core
DEG_INF = np.float32(1e38)
W = 4
WSZ = TR // W        # 25088 rows per gather window (int16-indexable)
CAP = 76             # max slab columns (64-elem units) per tile group


def _preprocess(edge_index, batch):
    src = np.asarray(edge_index[0], dtype=np.int64)
    dst = np.asarray(edge_index[1], dtype=np.int64)
    batch = np.asarray(batch, dtype=np.int64)

    core = np.arange(N, dtype=np.int64) // NPC_RAW
    indeg = np.bincount(dst, minlength=N).astype(np.int64)

    # per-node window profile (neighbors + self, by source core-pair)
    win_n = core // 2
    cw_n = np.zeros((N, W), dtype=np.int64)
    np.add.at(cw_n, (dst, win_n[src]), 1)
    cw_n[np.arange(N), win_n] += 1
    mx = cw_n.max(axis=1)

    # table id: group by core, sort by (max window count, degree) descending
    order = np.lexsort((np.arange(N), -(indeg + 1), -mx, core))
    core_sorted = core[order]
    block_start = np.searchsorted(core_sorted, np.arange(NC))
    within = np.arange(N, dtype=np.int64) - block_start[core_sorted]
    tid = np.empty(N, dtype=np.int64)
    tid[order] = core_sorted * NPC + within

    src_t = tid[src]
    dst_t = tid[dst]

    # per-row per-window counts (incl self)
    rows = np.arange(TR, dtype=np.int64)
    w_e = src_t // WSZ
    cw_row = np.zeros((TR, W), dtype=np.int64)
    np.add.at(cw_row, (dst_t, w_e), 1)
    w_s = rows // WSZ
    rank_s = cw_row[rows, w_s].copy()          # self slot = after edges
    cw_row[rows, w_s] += 1

    tile_of_row = (rows % NPC) // 128
    Kw_t = np.zeros((NT, W), dtype=np.int64)
    for t in range(NT):
        Kw_t[t] = cw_row[tile_of_row == t].max(axis=0)

    # greedy tile grouping with slab cap; Kw regularized per group
    groups = []
    t0 = 0
    while t0 < NT:
        t1 = t0 + 1
        Kw_g = Kw_t[t0].copy()
        while t1 < NT:
            cand = np.maximum(Kw_g, Kw_t[t1])
            if (t1 + 1 - t0) * int(cand.sum()) > CAP:
                break
            Kw_g = cand
            t1 += 1
        assert (t1 - t0) * int(Kw_g.sum()) <= CAP
        groups.append((t0, t1, Kw_g.astype(np.int64)))
        t0 = t1

    group_of_tile = np.zeros(NT, dtype=np.int64)
    ginfo = []
    goff = 0
    for gi, (a, b, kw) in enumerate(groups):
        group_of_tile[a:b] = gi
        gg = b - a
        skg = int(kw.sum())
        wbase = np.concatenate([[0], np.cumsum(kw)]) * gg   # col offsets per window
        woff = 8 * gg * np.concatenate([[0], np.cumsum(kw)])  # idx col offsets
        gcols = 8 * gg * skg
        ginfo.append(dict(t0=a, G=gg, Kw=[int(x) for x in kw], SKg=skg,
                          wbase=[int(x) for x in wbase[:-1]],
                          woff=[int(x) for x in woff[:-1]],
                          goff=goff, gcols=gcols))
        goff += gcols
    TOTC = goff

    # slot ranks per edge within (dst row, window)
    eorder = np.lexsort((src_t, w_e, dst_t))
    ds, ws_, ss = dst_t[eorder], w_e[eorder], src_t[eorder]
    grp_key = ds * W + ws_
    seg_start = np.searchsorted(grp_key, np.arange(TR * W))
    rank = np.arange(E, dtype=np.int64) - seg_start[grp_key]

    t0_of = np.array([ginfo[group_of_tile[t]]["t0"] for t in range(NT)])
    KwG = np.zeros((NT, W), dtype=np.int64)
    for t in range(NT):
        KwG[t] = ginfo[group_of_tile[t]]["Kw"]
    woff_t = np.zeros((NT, W), dtype=np.int64)
    goff_t = np.zeros(NT, dtype=np.int64)
    for t in range(NT):
        gi = group_of_tile[t]
        woff_t[t] = ginfo[gi]["woff"]
        goff_t[t] = ginfo[gi]["goff"]

    idxg = np.full((NC, 128, TOTC), ZPAD, dtype=np.int16)

    def fill(dst_rows, wvals, ranks, srcs):
        c_x = dst_rows // NPC
        row_x = dst_rows % NPC
        t_x = row_x // 128
        p_x = row_x % 128
        tloc = t_x - t0_of[t_x]
        j = tloc * KwG[t_x, wvals] + ranks
        pos = j * 128 + p_x
        col = goff_t[t_x] + woff_t[t_x, wvals] + pos // 16
        part = (pos % 16).astype(np.int64)
        val = (srcs - wvals * WSZ).astype(np.int16)
        for r in range(8):
            idxg[c_x, part + 16 * r, col] = val

    fill(ds, ws_, rank, ss)
    fill(rows, w_s, rank_s, rows)

    # degree (with self loop) per row, phantoms get DEG_INF so dinv ~ 0
    indeg_row = np.zeros(TR, dtype=np.int64)
    indeg_row[tid] = indeg
    deg_f = np.full(TR, DEG_INF, dtype=np.float32)
    real = (rows % NPC) < NPC_RAW
    deg_f[real] = (indeg_row[real] + 1).astype(np.float32)
    deg_f = deg_f.reshape(NC, NT, 128).transpose(0, 2, 1).copy()  # [NC,128,NT]

    # pooling: graphs blocked by id; members are contiguous in original order
    cnt = np.bincount(batch, minlength=G).astype(np.int64)
    gstart = np.concatenate([[0], np.cumsum(cnt)])
    KP = int(np.ceil(cnt.max() / 4)) if cnt.max() > 0 else 1
    idxP = np.full((NC, 128, KP), ZPAD, dtype=np.int32)
    npad = np.full((NC, 128, 1), float(KP), dtype=np.float32)
    member0 = np.full((NC, 128, 1), ZPAD, dtype=np.int32)
    cntinv = np.zeros((NC, GPC), dtype=np.float32)
    for g in range(G):
        c, gl = g // GPC, g % GPC
        mem = tid[gstart[g]:gstart[g + 1]]
        if len(mem) == 0:
            continue
        cntinv[c, gl] = 1.0 / len(mem)
        m0 = np.int32(mem[0])
        for q in range(4):
            p = 4 * gl + q
            mq = mem[q::4]
            idxP[c, p, :] = m0
            idxP[c, p, :len(mq)] = mq
            npad[c, p, 0] = KP - len(mq)
            member0[c, p, 0] = m0
    cntinv_fm = np.repeat(cntinv[:, None, :], D, axis=1).copy()  # [NC, 64, GPC]

    return dict(
        tid=tid, deg_f=deg_f, ginfo=ginfo, idxg=idxg, TOTC=TOTC,
        idxP=idxP, npad=npad, member0=member0, cntinv_fm=cntinv_fm, KP=KP,
    )


def _numpy_model(x, prep, Ws, gs, bes):
    """float32 mirror of the device algorithm (validation only)."""
    tid = prep["tid"]
    xp = np.zeros((TR, D), dtype=np.float32)
    xp[tid] = x
    h_fm = np.stack([xp[c * NPC:(c + 1) * NPC].T for c in range(NC)])  # [NC,64,NPC]
    deg = prep["deg_f"]                                   # [NC,128,NT]
    dinv = np.sqrt(np.float32(1.0) / deg).astype(np.float32)
    dinv_flat = dinv.transpose(0, 2, 1).reshape(NC, NPC)  # row-major per core
    ginfo, idxg = prep["ginfo"], prep["idxg"]

    for l in range(3):
        Wm, g_, be = Ws[l], gs[l], bes[l]
        table = np.zeros((TR, D), dtype=np.float32)
        for c in range(NC):
            t_fm = (Wm.T.astype(np.float32) @ h_fm[c]).astype(np.float32)
            hh = (t_fm.T * dinv_flat[c][:, None]).astype(np.float32)
            table[c * NPC:(c + 1) * NPC] = hh
        y_fm = np.zeros((NC, D, NPC), dtype=np.float32)
        ssum = np.zeros((NC, D), dtype=np.float32)
        sq = np.zeros((NC, D), dtype=np.float32)
        for c in range(NC):
            for gg in ginfo:
                t0, Gg, Kw, SKg = gg["t0"], gg["G"], gg["Kw"], gg["SKg"]
                slab = np.zeros((128, Gg * SKg, D), dtype=np.float32)
                for w in range(W):
                    if Kw[w] == 0:
                        continue
                    NIw = 128 * Gg * Kw[w]
                    # unwrap idx buffer
                    cols = slice(gg["goff"] + gg["woff"][w],
                                 gg["goff"] + gg["woff"][w] + 8 * Gg * Kw[w])
                    buf = idxg[c][:16, cols]
                    flat = buf.T.reshape(-1)[:NIw].astype(np.int64)
                    flat = np.where(flat < 0, flat + 65536, flat)
                    jb = gg["wbase"][w]
                    for i in range(NIw):
                        p, j = i % 128, i // 128
                        slab[p, jb + j] = table[w * WSZ + flat[i]]
                for tloc in range(Gg):
                    t = t0 + tloc
                    sl = slice(t * 128, (t + 1) * 128)
                    msum = np.zeros((128, D), dtype=np.float32)
                    for w in range(W):
                        if Kw[w] == 0:
                            continue
                        jb = gg["wbase"][w] + tloc * Kw[w]
                        msum += slab[:, jb:jb + Kw[w]].sum(axis=1, dtype=np.float32)
                    acc = (msum * dinv[c, :, t][:, None]).astype(np.float32)
                    y_fm[c][:, sl] = acc.T
            ssum[c] = y_fm[c].sum(axis=1, dtype=np.float32)
            sq[c] = (y_fm[c].astype(np.float32) ** 2).sum(axis=1, dtype=np.float32)
        S = ssum.sum(axis=0, dtype=np.float32)
        Q = sq.sum(axis=0, dtype=np.float32)
        mean = (S / np.float32(N)).astype(np.float32)
        var = (Q / np.float32(N) - mean * mean).astype(np.float32)
        rstd = np.sqrt(np.float32(1.0) / (var + np.float32(EPS))).astype(np.float32)
        scale = (g_ * rstd).astype(np.float32)
        shift = (be - mean * scale).astype(np.float32)
        for c in range(NC):
            h = (y_fm[c] * scale[:, None] + shift[:, None]).astype(np.float32)
            if l < 2:
                h = np.maximum(h, 0)
            h_fm[c] = h

    # pooling from full h3 table
    table = np.zeros((TR, D), dtype=np.float32)
    for c in range(NC):
        table[c * NPC:(c + 1) * NPC] = h_fm[c].T
    for c in range(NC):
        table[c * NPC + NPC_RAW:(c + 1) * NPC] = 0.0
    idxP, npad, member0, cntinv_fm = (
        prep["idxP"], prep["npad"], prep["member0"], prep["cntinv_fm"])
    out = np.zeros((G, D), dtype=np.float32)
    for c in range(NC):
        slab = table[idxP[c]]                      # [128, KP, 64]
        ssum = slab.sum(axis=1, dtype=np.float32)  # [128, 64]
        smax = slab.max(axis=1)
        m0row = table[member0[c, :, 0]]            # [128, 64]
        ssum = ssum - npad[c] * m0row
        q = ssum.reshape(GPC, 4, D).sum(axis=1, dtype=np.float32)   # [32, 64]
        m = smax.reshape(GPC, 4, D).max(axis=1)
        mean = q * cntinv_fm[c, 0][:, None]
        out[c * GPC:(c + 1) * GPC] = mean + m
    return out


def kernel(**inputs):
    x = np.asarray(inputs["x"], dtype=np.float32)
    prep = _preprocess(inputs["edge_index"], inputs["batch"])
    Ws = [np.asarray(inputs[f"W{i+1}"], dtype=np.float32) for i in range(3)]
    gs = [np.asarray(inputs[f"g{i+1}"], dtype=np.float32) for i in range(3)]
    bes = [np.asarray(inputs[f"be{i+1}"], dtype=np.float32) for i in range(3)]
    import os
    if os.environ.get("GCN_NUMPY_MODEL"):
        return _numpy_model(x, prep, Ws, gs, bes)
    return _run_device(x, prep, Ws, gs, bes)


HNPC = NPC // 2      # 6272 columns per half of the split feature-major layout
HT = NT // 2         # 49 tiles per half

_DEVICE_CACHE = {}


def _build_device(ginfo, TOTC, KP):
    import concourse.bacc as bacc
    import concourse.bass as bass
    import concourse.tile as tile
    import concourse.mybir as mybir
    from concourse.masks import make_identity
    from concourse.library_config import mlp

    fp32 = mybir.dt.float32
    i32 = mybir.dt.int32
    i16 = mybir.dt.int16
    GCOLS_MAX = max(g["gcols"] for g in ginfo)

    nc = bacc.Bacc("TRN2", target_bir_lowering=False, debug=False, num_devices=NC,
                   num_swdge_queues=4, dynamic_dma_scratch_size=32768)

    x_in = nc.dram_tensor("x_in", [128, HNPC], fp32, kind="ExternalInput")
    w_in = nc.dram_tensor("w_in", [3 * 64, 64], fp32, kind="ExternalInput")
    bn_in = nc.dram_tensor("bn_in", [6 * 64, 1], fp32, kind="ExternalInput")
    deg_in = nc.dram_tensor("deg_in", [128, NT], fp32, kind="ExternalInput")
    idxg_in = nc.dram_tensor("idxg_in", [128, TOTC], i16, kind="ExternalInput")
    idxp_in = nc.dram_tensor("idxp_in", [128, KP], i32, kind="ExternalInput")
    mem0_in = nc.dram_tensor("mem0_in", [128, 1], i32, kind="ExternalInput")
    npad_in = nc.dram_tensor("npad_in", [128, 1], fp32, kind="ExternalInput")
    phm_in = nc.dram_tensor("phm_in", [128, 1], fp32, kind="ExternalInput")
    cntinv_in = nc.dram_tensor("cntinv_in", [64, GPC], fp32, kind="ExternalInput")
    out_ext = nc.dram_tensor("out", [G, D], fp32, kind="ExternalOutput")

    slice_d = nc.dram_tensor("slice_d", [NPC, D], fp32)
    table_d = nc.dram_tensor("table_d", [TR, D], fp32, addr_space="Shared")
    stats_i = nc.dram_tensor("stats_i", [64, 2], fp32)
    stats_o = nc.dram_tensor("stats_o", [64, 2], fp32, addr_space="Shared")
    oslice_d = nc.dram_tensor("oslice_d", [GPC, D], fp32)
    ofull_d = nc.dram_tensor("ofull_d", [G, D], fp32, addr_space="Shared")

    RG = [list(range(NC))]
    INVN = 1.0 / float(N)

    with tile.TileContext(nc) as tc:
        with (
            tc.tile_pool(name="cp", bufs=1) as cp,
            tc.tile_pool(name="hp", bufs=2) as hp,
            tc.tile_pool(name="sp", bufs=3) as sp,
            tc.tile_pool(name="sm", bufs=4) as sm,
            tc.tile_pool(name="slb", bufs=3) as slb,
            tc.tile_pool(name="rp", bufs=2) as rp,
            tc.tile_pool(name="ixp", bufs=4) as ixp,
            tc.tile_pool(name="ps", bufs=2, space="PSUM") as ps,
        ):
            nc.gpsimd.load_library(mlp)
            ident = cp.tile([128, 128], fp32, tag="ident")
            make_identity(nc, ident[:])

            deg_sb = cp.tile([128, NT], fp32, tag="deg")
            nc.sync.dma_start(out=deg_sb[:], in_=deg_in[:])
            dinv = cp.tile([128, NT], fp32, tag="dinv")
            nc.vector.reciprocal(out=dinv[:], in_=deg_sb[:])
            nc.scalar.activation(out=dinv[:], in_=dinv[:],
                                 func=mybir.ActivationFunctionType.Sqrt)

            w_sb = []
            bn_sb = []

            def emit_body():
                h_cur = hp.tile([128, HNPC], fp32, tag="h")
                nc.sync.dma_start(out=h_cur[:], in_=x_in[:])
                for l in range(3):
                    wt = cp.tile([128, 64], fp32, tag=f"w{l}")
                    nc.sync.dma_start(out=wt[0:64, :], in_=w_in[l * 64:(l + 1) * 64, :])
                    nc.sync.dma_start(out=wt[64:128, :], in_=w_in[l * 64:(l + 1) * 64, :])
                    w_sb.append(wt)
                    bt = cp.tile([64, 2], fp32, tag=f"bn{l}")
                    nc.sync.dma_start(out=bt[:, 0:1], in_=bn_in[(2 * l) * 64:(2 * l + 1) * 64, :])
                    nc.sync.dma_start(out=bt[:, 1:2], in_=bn_in[(2 * l + 1) * 64:(2 * l + 2) * 64, :])
                    bn_sb.append(bt)

                hhat = cp.tile([128, NT * 64], fp32, tag="hhat")
                yst = cp.tile([128, HNPC], fp32, tag="yst")

                idxp_sb = cp.tile([128, KP], i32, tag="idxp")
                nc.sync.dma_start(out=idxp_sb[:], in_=idxp_in[:])
                mem0_sb = cp.tile([128, 1], i32, tag="mem0")
                nc.sync.dma_start(out=mem0_sb[:], in_=mem0_in[:])
                npad_sb = cp.tile([128, 1], fp32, tag="npad")
                nc.sync.dma_start(out=npad_sb[:], in_=npad_in[:])
                cinv_sb = cp.tile([64, GPC], fp32, tag="cinv")
                nc.sync.dma_start(out=cinv_sb[:], in_=cntinv_in[:])
                phm_sb = cp.tile([128, 1], fp32, tag="phm")
                nc.sync.dma_start(out=phm_sb[:], in_=phm_in[:])

                slice_v = slice_d[:].rearrange("(t p) d -> p t d", p=128)

                def emit_table_write_and_gather(src_nm):
                    # src_nm: [128, NT*64] staging of this core's table slice rows
                    nc.sync.dma_start(out=slice_v, in_=src_nm[:])
                    nc.gpsimd.collective_compute(
                        "AllGather", mybir.AluOpType.bypass, replica_groups=RG,
                        ins=[slice_d[:].opt()], outs=[table_d[:].opt()],
                    )

                for l in range(3):
                    # ---- GEMM + dinv scale -> hhat staging (table slice) ----
                    for t in range(NT):
                        half, c = divmod(t, HT)
                        pb = 64 * half
                        pt = ps.tile([64, 128], fp32, tag="pt", space="PSUM")
                        nc.tensor.matmul(
                            out=pt[:], lhsT=w_sb[l][pb:pb + 64, :],
                            rhs=h_cur[pb:pb + 64, c * 128:(c + 1) * 128],
                            start=True, stop=True,
                        )
                        stg = sp.tile([64, 128], fp32, tag="stg")
                        nc.scalar.activation(out=stg[:], in_=pt[:],
                                             func=mybir.ActivationFunctionType.Copy)
                        ptr = ps.tile([128, 64], fp32, tag="ptr", space="PSUM")
                        nc.tensor.transpose(out=ptr[:], in_=stg[:], identity=ident[0:64, 0:64])
                        nc.scalar.activation(
                            out=hhat[:, t * 64:(t + 1) * 64], in_=ptr[:],
                            func=mybir.ActivationFunctionType.Copy,
                            scale=dinv[:, t:t + 1])
                    emit_table_write_and_gather(hhat)

                    # ---- batched gathers + reduce -> yst ----
                    for gg in ginfo:
                        t0, Gg, Kw, SKg = gg["t0"], gg["G"], gg["Kw"], gg["SKg"]
                        idxt = ixp.tile([128, GCOLS_MAX], i16, tag="idxg")
                        nc.sync.dma_start(
                            out=idxt[:, 0:gg["gcols"]],
                            in_=idxg_in[:, gg["goff"]:gg["goff"] + gg["gcols"]])
                        slab = slb.tile([128, CAP * 64], fp32, tag="slab")
                        for w in range(W):
                            if Kw[w] == 0:
                                continue
                            NIw = 128 * Gg * Kw[w]
                            ob = gg["wbase"][w] * 64
                            outv = slab[:, ob:ob + Gg * Kw[w] * 64].rearrange(
                                "p (j d) -> p j d", d=64)
                            nc.gpsimd.dma_gather(
                                outv,
                                table_d[w * WSZ:(w + 1) * WSZ, :],
                                idxt[:, gg["woff"][w]:gg["woff"][w] + 8 * Gg * Kw[w]],
                                NIw, NIw, 64, single_packet=False, queue_num=w,
                            )
                        # per-window reduce over k -> contiguous partials, then
                        # one strided combine reduce
                        rws = rp.tile([128, W * Gg * 64], fp32, tag="rws")
                        nw = 0
                        for w in range(W):
                            if Kw[w] == 0:
                                continue
                            ob = gg["wbase"][w] * 64
                            dst = rws[:, nw * Gg * 64:(nw + 1) * Gg * 64]
                            if Kw[w] == 1:
                                nc.vector.tensor_copy(out=dst, in_=slab[:, ob:ob + Gg * 64])
                            else:
                                rin = slab[:, ob:ob + Gg * Kw[w] * 64].rearrange(
                                    "p (t k d) -> p t d k", t=Gg, k=Kw[w])
                                nc.vector.reduce_sum(out=dst, in_=rin,
                                                     axis=mybir.AxisListType.X)
                            nw += 1
                        msum = rp.tile([128, Gg * 64], fp32, tag="msum")
                        if nw == 1:
                            msum = rws
                        else:
                            cin = rws[:, 0:nw * Gg * 64].rearrange(
                                "p (w x) -> p x w", w=nw)
                            nc.vector.reduce_sum(out=msum[:], in_=cin,
                                                 axis=mybir.AxisListType.X)
                        msum3 = msum[:, 0:Gg * 64].rearrange("p (t d) -> p t d", d=64)
                        db = dinv[:, t0:t0 + Gg][:, :, None].broadcast_to(
                            [128, Gg, 64])
                        nc.vector.tensor_tensor(out=msum3, in0=msum3, in1=db,
                                                op=mybir.AluOpType.mult)
                        for tloc in range(Gg):
                            t = t0 + tloc
                            half, c = divmod(t, HT)
                            pb = 64 * half
                            pyt = ps.tile([64, 128], fp32, tag="pyt", space="PSUM")
                            nc.tensor.transpose(
                                out=pyt[:], in_=msum[:, tloc * 64:(tloc + 1) * 64],
                                identity=ident[:, 0:128])
                            nc.scalar.activation(
                                out=yst[pb:pb + 64, c * 128:(c + 1) * 128], in_=pyt[:],
                                func=mybir.ActivationFunctionType.Copy)

                    # ---- BN stats (sum, sumsq) over both halves ----
                    stt = sm.tile([128, 2], fp32, tag="stt")
                    sqs = cp.tile([64, 512], fp32, tag="sqs")
                    parts2 = []
                    for half in range(2):
                        pb = 64 * half
                        srow = sm.tile([128, 1], fp32, tag=f"srow{half}")
                        nc.vector.reduce_sum(out=srow[0:64, :], in_=yst[pb:pb + 64, :],
                                             axis=mybir.AxisListType.X)
                        qacc = sm.tile([128, 16], fp32, tag=f"qacc{half}")
                        nchunk = (HNPC + 511) // 512
                        for j in range(nchunk):
                            lo = j * 512
                            hi = min(lo + 512, HNPC)
                            nc.vector.tensor_tensor(
                                out=sqs[:, 0:hi - lo], in0=yst[pb:pb + 64, lo:hi],
                                in1=yst[pb:pb + 64, lo:hi], op=mybir.AluOpType.mult)
                            nc.vector.reduce_sum(
                                out=qacc[0:64, j:j + 1], in_=sqs[:, 0:hi - lo],
                                axis=mybir.AxisListType.X)
                        qsum = sm.tile([128, 1], fp32, tag=f"qsum{half}")
                        nc.vector.reduce_sum(out=qsum[0:64, :], in_=qacc[0:64, 0:nchunk],
                                             axis=mybir.AxisListType.X)
                        parts2.append((srow, qsum))
                    nc.vector.tensor_tensor(out=stt[0:64, 0:1], in0=parts2[0][0][0:64, :],
                                            in1=parts2[1][0][0:64, :], op=mybir.AluOpType.add)
                    nc.vector.tensor_tensor(out=stt[0:64, 1:2], in0=parts2[0][1][0:64, :],
                                            in1=parts2[1][1][0:64, :], op=mybir.AluOpType.add)
                    nc.sync.dma_start(out=stats_i[:], in_=stt[0:64, :])
                    nc.gpsimd.collective_compute(
                        "AllReduce", mybir.AluOpType.add, replica_groups=RG,
                        ins=[stats_i[:].opt()], outs=[stats_o[:].opt()],
                    )
                    stin = sm.tile([64, 2], fp32, tag="stin")
                    nc.sync.dma_start(out=stin[:], in_=stats_o[:])

                    # ---- BN coefficients ----
                    co = sm.tile([64, 8], fp32, tag="co")
                    mean, ex2, m2, var, rec, rstd = (co[:, i:i + 1] for i in range(6))
                    nc.vector.tensor_scalar_mul(out=mean, in0=stin[:, 0:1], scalar1=INVN)
                    nc.vector.tensor_scalar_mul(out=ex2, in0=stin[:, 1:2], scalar1=INVN)
                    nc.vector.tensor_tensor(out=m2, in0=mean, in1=mean, op=mybir.AluOpType.mult)
                    nc.vector.tensor_tensor(out=var, in0=ex2, in1=m2, op=mybir.AluOpType.subtract)
                    nc.vector.tensor_scalar_add(out=var, in0=var, scalar1=float(EPS))
                    nc.vector.reciprocal(out=rec, in_=var)
                    nc.scalar.activation(out=rstd, in_=rec, func=mybir.ActivationFunctionType.Sqrt)
                    scsh = sm.tile([128, 2], fp32, tag="scsh")
                    nc.vector.tensor_tensor(out=scsh[0:64, 0:1], in0=bn_sb[l][:, 0:1],
                                            in1=rstd, op=mybir.AluOpType.mult)
                    ms = co[:, 6:7]
                    nc.vector.tensor_tensor(out=ms, in0=mean, in1=scsh[0:64, 0:1],
                                            op=mybir.AluOpType.mult)
                    nc.vector.tensor_tensor(out=scsh[0:64, 1:2], in0=bn_sb[l][:, 1:2],
                                            in1=ms, op=mybir.AluOpType.subtract)
                    nc.vector.tensor_copy(out=scsh[64:128, :], in_=scsh[0:64, :])

                    # ---- BN apply (+ReLU) -> next h ----
                    h_nxt = hp.tile([128, HNPC], fp32, tag="h")
                    for half in range(2):
                        pb = 64 * half
                        if l < 2:
                            nc.scalar.activation(
                                out=h_nxt[pb:pb + 64, :], in_=yst[pb:pb + 64, :],
                                func=mybir.ActivationFunctionType.Relu,
                                bias=scsh[pb:pb + 64, 1:2], scale=scsh[pb:pb + 64, 0:1])
                        else:
                            nc.vector.tensor_scalar(
                                out=h_nxt[pb:pb + 64, :], in0=yst[pb:pb + 64, :],
                                scalar1=scsh[pb:pb + 64, 0:1], scalar2=scsh[pb:pb + 64, 1:2],
                                op0=mybir.AluOpType.mult, op1=mybir.AluOpType.add)
                    h_cur = h_nxt

                # ---- h3 -> table ----
                for t in range(NT):
                    half, c = divmod(t, HT)
                    pb = 64 * half
                    ph = ps.tile([128, 64], fp32, tag="ptr", space="PSUM")
                    nc.tensor.transpose(out=ph[:], in_=h_cur[pb:pb + 64, c * 128:(c + 1) * 128],
                                        identity=ident[pb:pb + 64, pb:pb + 64])
                    nc.scalar.activation(out=hhat[:, t * 64:(t + 1) * 64], in_=ph[:],
                                         func=mybir.ActivationFunctionType.Copy)
                nc.vector.tensor_scalar_mul(
                    out=hhat[:, (NT - 1) * 64:NT * 64],
                    in0=hhat[:, (NT - 1) * 64:NT * 64], scalar1=phm_sb[:, 0:1])
                emit_table_write_and_gather(hhat)

                # ---- pooling (chunks of CAP columns, reusing slb pool) ----
                m0row = sm.tile([128, 64], fp32, tag="m0row")
                nc.gpsimd.indirect_dma_start(
                    out=m0row[:], out_offset=None, in_=table_d[:],
                    in_offset=bass.IndirectOffsetOnAxis(ap=mem0_sb[:, 0:1], axis=0),
                )
                ssum = sm.tile([128, 64], fp32, tag="ssum")
                smax = sm.tile([128, 64], fp32, tag="smax")
                for k0 in range(0, KP, CAP):
                    kn = min(CAP, KP - k0)
                    slabp = slb.tile([128, CAP * 64], fp32, tag="slab")
                    for k in range(kn):
                        nc.gpsimd.indirect_dma_start(
                            out=slabp[:, k * 64:(k + 1) * 64], out_offset=None,
                            in_=table_d[:],
                            in_offset=bass.IndirectOffsetOnAxis(
                                ap=idxp_sb[:, k0 + k:k0 + k + 1], axis=0),
                        )
                    pv = slabp[:, 0:kn * 64].rearrange("p (k d) -> p d k", k=kn)
                    if k0 == 0:
                        nc.vector.reduce_sum(out=ssum[:], in_=pv,
                                             axis=mybir.AxisListType.X)
                        nc.vector.reduce_max(out=smax[:], in_=pv,
                                             axis=mybir.AxisListType.X)
                    else:
                        ts_ = sm.tile([128, 64], fp32, tag="tsum")
                        tm_ = sm.tile([128, 64], fp32, tag="tmax")
                        nc.vector.reduce_sum(out=ts_[:], in_=pv,
                                             axis=mybir.AxisListType.X)
                        nc.vector.reduce_max(out=tm_[:], in_=pv,
                                             axis=mybir.AxisListType.X)
                        nc.vector.tensor_tensor(out=ssum[:], in0=ssum[:], in1=ts_[:],
                                                op=mybir.AluOpType.add)
                        nc.vector.tensor_tensor(out=smax[:], in0=smax[:], in1=tm_[:],
                                                op=mybir.AluOpType.max)
                corr = sm.tile([128, 64], fp32, tag="corr")
                nc.vector.tensor_scalar_mul(out=corr[:], in0=m0row[:], scalar1=npad_sb[:, 0:1])
                nc.vector.tensor_tensor(out=ssum[:], in0=ssum[:], in1=corr[:],
                                        op=mybir.AluOpType.subtract)

                def to_fm(src, tg):
                    p = ps.tile([64, 128], fp32, tag="pyt", space="PSUM")
                    nc.tensor.transpose(out=p[:], in_=src[:], identity=ident[:, 0:128])
                    t = sm.tile([64, 128], fp32, tag="fm" + tg)
                    nc.vector.tensor_copy(out=t[:], in_=p[:])
                    return t

                sfm = to_fm(ssum, "s")
                mfm = to_fm(smax, "m")

                def qcombine(t, op, tg):
                    v = t[:].rearrange("f (g q) -> f q g", q=4)
                    a = sm.tile([64, GPC], fp32, tag="qa" + tg)
                    b = sm.tile([64, GPC], fp32, tag="qb" + tg)
                    nc.vector.tensor_tensor(out=a[:], in0=v[:, 0, :], in1=v[:, 1, :], op=op)
                    nc.vector.tensor_tensor(out=b[:], in0=v[:, 2, :], in1=v[:, 3, :], op=op)
                    nc.vector.tensor_tensor(out=a[:], in0=a[:], in1=b[:], op=op)
                    return a

                s32 = qcombine(sfm, mybir.AluOpType.add, "s")
                m32 = qcombine(mfm, mybir.AluOpType.max, "m")
                outfm = sm.tile([64, GPC], fp32, tag="outfm")
                nc.vector.tensor_tensor(out=outfm[:], in0=s32[:], in1=cinv_sb[:],
                                        op=mybir.AluOpType.mult)
                nc.vector.tensor_tensor(out=outfm[:], in0=outfm[:], in1=m32[:],
                                        op=mybir.AluOpType.add)
                po = ps.tile([GPC, 64], fp32, tag="ptr", space="PSUM")
                nc.tensor.transpose(out=po[:], in_=outfm[:], identity=ident[0:64, 0:64])
                onm = sm.tile([GPC, 64], fp32, tag="onm")
                nc.vector.tensor_copy(out=onm[:], in_=po[:])
                nc.sync.dma_start(out=oslice_d[:], in_=onm[:])
                nc.gpsimd.collective_compute(
                    "AllGather", mybir.AluOpType.bypass, replica_groups=RG,
                    ins=[oslice_d[:].opt()], outs=[ofull_d[:].opt()],
                )
                for half in range(2):
                    ot = sm.tile([128, 64], fp32, tag="ot")
                    nc.sync.dma_start(out=ot[:], in_=ofull_d[half * 128:(half + 1) * 128, :])
                    nc.sync.dma_start(out=out_ext[half * 128:(half + 1) * 128, :], in_=ot[:])

            emit_body()

    nc.compile()
    return nc


_PHMASK = (np.arange(12416, 12544)[:, None] < NPC_RAW).astype(np.float32)


def _make_inmaps(x, prep, Ws, gs, bes):
    tid = prep["tid"]
    xp = np.zeros((TR, D), dtype=np.float32)
    xp[tid] = x
    w_np = np.concatenate(Ws, axis=0).astype(np.float32)          # [192, 64]
    bn_np = np.zeros((6 * 64, 1), dtype=np.float32)
    for l in range(3):
        bn_np[(2 * l) * 64:(2 * l + 1) * 64, 0] = gs[l]
        bn_np[(2 * l + 1) * 64:(2 * l + 2) * 64, 0] = bes[l]
    in_maps = []
    for c in range(NC):
        sl = xp[c * NPC:(c + 1) * NPC]                            # [NPC, 64]
        xs = np.zeros((128, HNPC), dtype=np.float32)
        xs[0:64, :] = sl[:HNPC].T
        xs[64:128, :] = sl[HNPC:].T
        in_maps.append({
            "x_in": xs,
            "w_in": w_np,
            "bn_in": bn_np,
            "deg_in": prep["deg_f"][c],
            "idxg_in": prep["idxg"][c],
            "idxp_in": prep["idxP"][c],
            "mem0_in": prep["member0"][c],
            "npad_in": prep["npad"][c],
            "phm_in": _PHMASK,
            "cntinv_in": prep["cntinv_fm"][c],
        })
    return in_maps


def _run_device(x, prep, Ws, gs, bes):
    from concourse.bass_utils import run_bass_kernel_spmd

    import os
    key = (prep["TOTC"], prep["KP"],
           tuple(tuple(g["Kw"]) for g in prep["ginfo"]))
    if key not in _DEVICE_CACHE:
        _DEVICE_CACHE[key] = _build_device(prep["ginfo"], prep["TOTC"], prep["KP"])
    nc = _DEVICE_CACHE[key]
    in_maps = _make_inmaps(x, prep, Ws, gs, bes)
    trace = bool(os.environ.get("GCN_TRACE"))
    kw = {}
    if trace:
        kw["trace"] = True
        td = os.environ.get("GCN_TRACE_DIR")
        if td:
            os.makedirs(td, exist_ok=True)
            kw["tmpdir"] = td
    res = run_bass_kernel_spmd(nc, in_maps, core_ids=list(range(NC)), **kw)
    global _LAST_RES
    _LAST_RES = res
    return np.asarray(res.results[0]["out"], dtype=np.float32)



# revision 4
# speedup vs baseline: 1.4315x; 1.4315x over previous
"""GCN feature extractor on 8 Trainium2 NeuronCores — v2.

Distribution: nodes block-sharded over 8 cores (12500 each, padded to 12544).
Each core's rows are split into 4 "quarters" (3200/3200/3200/2944 rows); the
gather table is organized as 4 windows, window w = concat over cores of their
quarter-w rows (<= 25600 rows, int16-addressable). A greedy balancer assigns
nodes to quarters so each destination's in-edges spread evenly over the 4
windows, minimizing slab padding. Per layer: transpose-free GEMM (nodes on
PSUM partitions) + dinv scale -> 4 quarter AllGathers (pipelined) -> batched
dma_gather per (tile-group, window) -> strided reduces + local self-term add.
BatchNorm stats via tiny AllReduce. Mean+max pooling via per-window batched
gathers of graph members; final [256,64] assembled with an AllGather.
"""

import numpy as np

N = 100000
E = 1600000
D = 64
G = 256
NC = 8
NPC_RAW = 12500
NPC = 12544          # 98 tiles of 128
NT = NPC // 128      # 98
EPS = 1e-5
GPC = G // NC        # 32 graphs per core
DEG_INF = np.float32(1e38)
W = 4
QSZ = [3200, 3200, 3200, 2944]        # rows per (core, quarter)
REAL = [3199, 3199, 3199, 2903]       # non-reserved rows per (core, quarter)
QB = [0, 3200, 6400, 9600]            # local block starts
WSZL = [8 * s for s in QSZ]           # window sizes
CAP = 48                              # max slab columns (64-elem units) per group
HNPC = NPC // 2
_PHM_TILES = [24, 49, 74, 97]         # tiles containing reserved zero rows


def _zero_masks():
    phm = np.ones((128, 4), dtype=np.float32)
    phm[127, 0] = 0.0   # q0 zero row: l=3199 -> t24 p127
    phm[127, 1] = 0.0   # q1: l=6399 -> t49 p127
    phm[127, 2] = 0.0   # q2: l=9599 -> t74 p127
    phm[87:, 3] = 0.0   # q3: l=12503..12543 -> t97 p87..127
    return phm


def _balance_quarters(src, dst, core, outdeg):
    """Assign each node a quarter in {0..3} balancing per-dst window counts."""
    order_e = np.argsort(src, kind="stable")
    dst_by_src = dst[order_e]
    ptr = np.concatenate([[0], np.cumsum(outdeg)])
    cnt = np.zeros((N, W), dtype=np.int32)
    quota = np.tile(np.array(REAL, dtype=np.int64), (NC, 1))
    node_order = np.argsort(-outdeg, kind="stable")
    q_of = np.full(N, -1, dtype=np.int8)
    for u in node_order:
        ds = dst_by_src[ptr[u]:ptr[u + 1]]
        c = core[u]
        if len(ds) == 0:
            q = int(np.argmax(quota[c]))
        else:
            cv = cnt[ds]
            score = (2 * cv.astype(np.int64) + 1).sum(axis=0)
            score = score + (quota[c] <= 0) * (1 << 40)
            q = int(np.argmin(score))
            cnt[ds, q] += 1
        q_of[u] = q
        quota[c, q] -= 1
    return q_of, cnt


def _pack_cols(dst_part, wvals, ranks, vals, Kw_c, woff_c, goff_c, idx_arr):
    """Shared idx packing: slot (j*128+p) -> col = goff+woff+pos//16,
    part rows pos%16 + 16r."""
    j = ranks
    pos = j * 128 + dst_part
    col = goff_c + woff_c + pos // 16
    part = (pos % 16).astype(np.int64)
    for r in range(8):
        idx_arr[part + 16 * r, col] = vals


def _preprocess(edge_index, batch):
    src = np.asarray(edge_index[0], dtype=np.int64)
    dst = np.asarray(edge_index[1], dtype=np.int64)
    batch = np.asarray(batch, dtype=np.int64)

    core = np.arange(N, dtype=np.int64) // NPC_RAW
    indeg = np.bincount(dst, minlength=N).astype(np.int64)
    outdeg = np.bincount(src, minlength=N).astype(np.int64)

    q_of, cnt = _balance_quarters(src, dst, core, outdeg)

    # local position: within (core, quarter), sort by (-max cnt, vector, -indeg)
    lpos = np.empty(N, dtype=np.int64)    # local row l in [0, NPC)
    for c in range(NC):
        for q in range(W):
            sel = np.where((core == c) & (q_of == q))[0]
            cv = cnt[sel]
            o = np.lexsort((sel, -cv[:, 3], -cv[:, 2], -cv[:, 1], -cv[:, 0],
                            -cv.max(axis=1)))
            lpos[sel[o]] = QB[q] + np.arange(len(sel))

    qq = q_of.astype(np.int64)
    kk = lpos - np.array(QB)[qq]
    widx = core * np.array(QSZ)[qq] + kk          # window-local index (int16)
    WBq = np.concatenate([[0], np.cumsum(WSZL)])
    tid = WBq[qq] + widx                          # global table row

    # per-edge: destination (tile, part), window = quarter(src)
    dst_l = lpos[dst]
    t_e = dst_l // 128
    p_e = dst_l % 128
    c_e = core[dst]
    w_e = qq[src]

    # ranks within (dst, window)
    key = ((c_e * NPC + dst_l) * W + w_e)
    eorder = np.lexsort((src, key))
    ks = key[eorder]
    seg_start = np.searchsorted(ks, np.arange(NC * NPC * W))
    rank = np.arange(E, dtype=np.int64) - seg_start[ks]

    # per (core, tile, window) K = max count
    cw_row = np.zeros((NC * NPC, W), dtype=np.int64)
    np.add.at(cw_row, (c_e * NPC + dst_l, w_e), 1)
    KT = np.zeros((NC, NT, W), dtype=np.int64)
    rows_l = np.arange(NC * NPC)
    for w in range(W):
        np.maximum.at(KT[:, :, w].reshape(-1),
                      (rows_l // NPC) * NT + (rows_l % NPC) // 128, cw_row[:, w])

    # greedy tile grouping (same structure for all cores: use per-core maxes? no,
    # groups must be identical across cores for SPMD -> take max over cores)
    KTm = KT.max(axis=0)                          # [NT, W]
    groups = []
    t0 = 0
    while t0 < NT:
        t1 = t0 + 1
        Kg = KTm[t0].copy()
        while t1 < NT:
            cand = np.maximum(Kg, KTm[t1])
            if (t1 + 1 - t0) * int(cand.sum()) > CAP:
                break
            Kg = cand
            t1 += 1
        assert (t1 - t0) * int(Kg.sum()) <= CAP
        groups.append((t0, t1, Kg.astype(np.int64)))
        t0 = t1

    ginfo = []
    goff = 0
    group_of_tile = np.zeros(NT, dtype=np.int64)
    for gi, (a, b, kw) in enumerate(groups):
        group_of_tile[a:b] = gi
        gg = b - a
        skg = int(kw.sum())
        wbase = np.concatenate([[0], np.cumsum(kw)]) * gg
        woff = 8 * gg * np.concatenate([[0], np.cumsum(kw)])
        gcols = 8 * gg * skg
        ginfo.append(dict(t0=a, G=gg, Kw=[int(x) for x in kw], SKg=skg,
                          wbase=[int(x) for x in wbase[:-1]],
                          woff=[int(x) for x in woff[:-1]],
                          goff=goff, gcols=gcols))
        goff += gcols
    TOTC = goff

    # zero-row (per core, per window) window-local index for padding slots
    zidx = np.zeros((NC, W), dtype=np.int64)
    for c in range(NC):
        for w in range(W):
            zidx[c, w] = c * QSZ[w] + REAL[w]

    t0_of = np.array([ginfo[group_of_tile[t]]["t0"] for t in range(NT)])
    KwG = np.zeros((NT, W), dtype=np.int64)
    woff_t = np.zeros((NT, W), dtype=np.int64)
    goff_t = np.zeros(NT, dtype=np.int64)
    for t in range(NT):
        gi = group_of_tile[t]
        KwG[t] = ginfo[gi]["Kw"]
        woff_t[t] = ginfo[gi]["woff"]
        goff_t[t] = ginfo[gi]["goff"]

    idxg = np.empty((NC, 128, TOTC), dtype=np.int16)
    for c in range(NC):
        for w in range(W):
            # fill pad default per (c, w): columns of window w across groups
            for gg in ginfo:
                if gg["Kw"][w] == 0:
                    continue
                lo = gg["goff"] + gg["woff"][w]
                hi = lo + 8 * gg["G"] * gg["Kw"][w]
                idxg[c, :, lo:hi] = np.int16(zidx[c, w])
    # real edges
    tloc = t_e - t0_of[t_e]
    rank_orig = np.empty(E, dtype=np.int64)
    rank_orig[eorder] = rank
    jslot = tloc * KwG[t_e, w_e] + rank_orig
    pos = jslot * 128 + p_e
    col = goff_t[t_e] + woff_t[t_e, w_e] + pos // 16
    part = (pos % 16).astype(np.int64)
    val = widx[src].astype(np.int16)
    for r in range(8):
        idxg[c_e, part + 16 * r, col] = val

    # degree (with self loop); reserved rows get INF so dinv ~ 0
    deg_f = np.full((NC, NPC), DEG_INF, dtype=np.float32)
    deg_f[core, lpos] = (indeg + 1).astype(np.float32)
    deg_f = deg_f.reshape(NC, NT, 128).transpose(0, 2, 1).copy()   # [NC,128,NT]

    # ---- pooling: per-window batched gathers ----
    cnt_g = np.bincount(batch, minlength=G).astype(np.int64)
    gstart = np.concatenate([[0], np.cumsum(cnt_g)])
    # members per (core, p=4*gl+q_lane, window)
    memb = [[[[] for _ in range(W)] for _ in range(128)] for _ in range(NC)]
    for g in range(G):
        c, gl = g // GPC, g % GPC
        mem = np.arange(gstart[g], gstart[g + 1])
        for qlane in range(4):
            p = 4 * gl + qlane
            for u in mem[qlane::4]:
                w = qq[u]
                memb[c][p][w].append(widx[u])
    KPW = np.zeros(W, dtype=np.int64)
    for w in range(W):
        KPW[w] = max(1, max(len(memb[c][p][w]) for c in range(NC)
                            for p in range(128)))
    assert max(KPW) <= CAP, KPW
    idxP = []
    hasw = np.zeros((NC, 128, W), dtype=np.float32)
    npadw = np.zeros((NC, 128, W), dtype=np.float32)
    for w in range(W):
        a = np.empty((NC, 128, int(KPW[w])), dtype=np.int16)
        for c in range(NC):
            a[c, :, :] = np.int16(zidx[c, w])
            for p in range(128):
                m = memb[c][p][w]
                if m:
                    a[c, p, :] = np.int16(m[0])   # pads = first member (max-safe)
                    a[c, p, :len(m)] = np.asarray(m, dtype=np.int16)
                    hasw[c, p, w] = 1.0
                    npadw[c, p, w] = float(KPW[w] - len(m))
        idxP.append(a)
    hofs = ((1.0 - hasw) * np.float32(-1e30)).astype(np.float32)
    # pack pooling indices in gather layout: [NC, 128, sum_w 8*KPW]
    pw_off = np.concatenate([[0], np.cumsum(8 * KPW)])
    idxPg = np.empty((NC, 128, int(pw_off[-1])), dtype=np.int16)
    for w in range(W):
        kw = int(KPW[w])
        for c in range(NC):
            flat = idxP[w][c].T.reshape(-1)        # [kw*128] in (j, p) order
            buf = flat.reshape(kw * 8, 16).T       # [16, kw*8]
            idxPg[c, :, pw_off[w]:pw_off[w + 1]] = np.tile(buf, (8, 1))
    any_m = hasw.max(axis=2)                       # [NC, 128]
    cntinv = np.zeros((NC, GPC), dtype=np.float32)
    for g in range(G):
        c, gl = g // GPC, g % GPC
        if cnt_g[g] > 0:
            cntinv[c, gl] = 1.0 / cnt_g[g]
    cntinv_fm = np.repeat(cntinv[:, None, :], D, axis=1).copy()

    return dict(
        tid=tid, lpos=lpos, deg_f=deg_f, ginfo=ginfo, idxg=idxg, TOTC=TOTC,
        idxPg=idxPg, KPW=[int(x) for x in KPW],
        pw_off=[int(x) for x in pw_off], hasw=hasw, hofs=hofs, npadw=npadw,
        any_m=any_m, cntinv_fm=cntinv_fm, qq=qq, widx=widx,
    )


def _numpy_model(x, prep, Ws, gs, bes):
    """float32 mirror of the device algorithm (validation only)."""
    lpos = prep["lpos"]
    deg = prep["deg_f"]                                    # [NC,128,NT]
    dinv = np.sqrt(np.float32(1.0) / deg).astype(np.float32)
    ginfo, idxg = prep["ginfo"], prep["idxg"]
    core = np.arange(N) // NPC_RAW

    # local x: [NC, 64, NPC] feature-major
    h_fm = np.zeros((NC, D, NPC), dtype=np.float32)
    h_fm[core, :, lpos] = x
    windows = [np.zeros((WSZL[w], D), dtype=np.float32) for w in range(W)]

    def stage_tables(hh_nm):
        # hh_nm: [NC, NPC, D] node-major table slices (dinv-scaled)
        for w in range(W):
            for c in range(NC):
                windows[w][c * QSZ[w]:(c + 1) * QSZ[w]] = (
                    hh_nm[c][QB[w]:QB[w] + QSZ[w]])

    for l in range(3):
        Wm, g_, be = Ws[l], gs[l], bes[l]
        hh_nm = np.zeros((NC, NPC, D), dtype=np.float32)
        hhat_s = np.zeros((NC, NPC, D), dtype=np.float32)
        for c in range(NC):
            t_fm = (Wm.T.astype(np.float32) @ h_fm[c]).astype(np.float32)
            dflat = dinv[c].T.reshape(-1)
            hh = (t_fm.T * dflat[:, None]).astype(np.float32)
            hh_nm[c] = hh
            hhat_s[c] = hh
        stage_tables(hh_nm)
        y_nm = np.zeros((NC, NPC, D), dtype=np.float32)
        for c in range(NC):
            for gg in ginfo:
                t0, Gg, Kw = gg["t0"], gg["G"], gg["Kw"]
                msum = np.zeros((128, Gg, D), dtype=np.float32)
                for w in range(W):
                    if Kw[w] == 0:
                        continue
                    NIw = 128 * Gg * Kw[w]
                    lo = gg["goff"] + gg["woff"][w]
                    buf = idxg[c][:16, lo:lo + 8 * Gg * Kw[w]]
                    flat = buf.T.reshape(-1)[:NIw].astype(np.int64)
                    flat = np.where(flat < 0, flat + 65536, flat)
                    ii = np.arange(NIw)
                    np.add.at(msum, (ii % 128, (ii // 128) // Kw[w]),
                              windows[w][flat])
                for tl in range(Gg):
                    t = t0 + tl
                    sl = slice(t * 128, (t + 1) * 128)
                    acc = (msum[:, tl] + hhat_s[c][sl]) * dinv[c, :, t][:, None]
                    y_nm[c][sl] = acc.astype(np.float32)
        S = y_nm.sum(axis=(0, 1), dtype=np.float32)
        Q = (y_nm.astype(np.float32) ** 2).sum(axis=(0, 1), dtype=np.float32)
        mean = (S / np.float32(N)).astype(np.float32)
        var = (Q / np.float32(N) - mean * mean).astype(np.float32)
        rstd = np.sqrt(np.float32(1.0) / (var + np.float32(EPS))).astype(np.float32)
        scale = (g_ * rstd).astype(np.float32)
        shift = (be - mean * scale).astype(np.float32)
        for c in range(NC):
            h = (y_nm[c] * scale[None, :] + shift[None, :]).astype(np.float32)
            if l < 2:
                h = np.maximum(h, 0)
            h_fm[c] = h.T

    # final table (zero rows forced to exactly 0)
    phm = _zero_masks()
    h3_nm = np.zeros((NC, NPC, D), dtype=np.float32)
    for c in range(NC):
        h3_nm[c] = h_fm[c].T
        for qi, t in enumerate(_PHM_TILES):
            mask = phm[:, qi:qi + 1]
            h3_nm[c][t * 128:(t + 1) * 128] *= mask
    stage_tables(h3_nm)

    idxPg, KPW, pw_off = prep["idxPg"], prep["KPW"], prep["pw_off"]
    hasw, hofs, npadw = prep["hasw"], prep["hofs"], prep["npadw"]
    any_m, cntinv_fm = prep["any_m"], prep["cntinv_fm"]
    out = np.zeros((G, D), dtype=np.float32)
    for c in range(NC):
        ssum = np.zeros((128, D), dtype=np.float32)
        smax = np.full((128, D), -np.inf, dtype=np.float32)
        for w in range(W):
            kw = KPW[w]
            buf = idxPg[c][:16, pw_off[w]:pw_off[w + 1]]
            flat = buf.T.reshape(-1).astype(np.int64)
            flat = np.where(flat < 0, flat + 65536, flat)
            slab = windows[w][flat].reshape(kw, 128, D).transpose(1, 0, 2)
            ts = slab.sum(axis=1, dtype=np.float32)
            ts -= npadw[c, :, w:w + 1] * slab[:, 0, :]
            ssum += ts
            pm = slab.max(axis=1)
            adj = pm * hasw[c, :, w:w + 1] + hofs[c, :, w:w + 1]
            smax = np.maximum(smax, adj)
        smax = smax * any_m[c][:, None]
        q = ssum.reshape(GPC, 4, D).sum(axis=1, dtype=np.float32)
        m = smax.reshape(GPC, 4, D).max(axis=1)
        mean = q * cntinv_fm[c, 0][:, None]
        out[c * GPC:(c + 1) * GPC] = mean + m
    return out


def kernel(**inputs):
    x = np.asarray(inputs["x"], dtype=np.float32)
    prep = _preprocess(inputs["edge_index"], inputs["batch"])
    Ws = [np.asarray(inputs[f"W{i+1}"], dtype=np.float32) for i in range(3)]
    gs = [np.asarray(inputs[f"g{i+1}"], dtype=np.float32) for i in range(3)]
    bes = [np.asarray(inputs[f"be{i+1}"], dtype=np.float32) for i in range(3)]
    import os
    if os.environ.get("GCN_NUMPY_MODEL"):
        return _numpy_model(x, prep, Ws, gs, bes)
    return _run_device(x, prep, Ws, gs, bes)


_DEVICE_CACHE = {}


def _build_device(ginfo, TOTC, KPW, pw_off):
    import concourse.bacc as bacc
    import concourse.bass as bass
    import concourse.tile as tile
    import concourse.mybir as mybir
    from concourse.masks import make_identity
    from concourse.library_config import mlp

    fp32 = mybir.dt.float32
    i16 = mybir.dt.int16
    GCOLS_MAX = max(g["gcols"] for g in ginfo)

    nc = bacc.Bacc("TRN2", target_bir_lowering=False, debug=False, num_devices=NC,
                   num_swdge_queues=4, dynamic_dma_scratch_size=32768)

    x_in = nc.dram_tensor("x_in", [128, HNPC], fp32, kind="ExternalInput")
    w_in = nc.dram_tensor("w_in", [3 * 64, 64], fp32, kind="ExternalInput")
    bn_in = nc.dram_tensor("bn_in", [6 * 64, 1], fp32, kind="ExternalInput")
    deg_in = nc.dram_tensor("deg_in", [128, NT], fp32, kind="ExternalInput")
    idxg_in = nc.dram_tensor("idxg_in", [128, TOTC], i16, kind="ExternalInput")
    idxp_in = nc.dram_tensor("idxp_in", [128, pw_off[-1]], i16, kind="ExternalInput")
    hasw_in = nc.dram_tensor("hasw_in", [128, W], fp32, kind="ExternalInput")
    hofs_in = nc.dram_tensor("hofs_in", [128, W], fp32, kind="ExternalInput")
    npad_in = nc.dram_tensor("npad_in", [128, W], fp32, kind="ExternalInput")
    anym_in = nc.dram_tensor("anym_in", [128, 1], fp32, kind="ExternalInput")
    phm_in = nc.dram_tensor("phm_in", [128, 4], fp32, kind="ExternalInput")
    cntinv_in = nc.dram_tensor("cntinv_in", [64, GPC], fp32, kind="ExternalInput")
    out_ext = nc.dram_tensor("out", [G, D], fp32, kind="ExternalOutput")

    slice_q = [nc.dram_tensor(f"slice_q{w}", [QSZ[w], D], fp32) for w in range(W)]
    win_d = [nc.dram_tensor(f"win_d{w}", [WSZL[w], D], fp32, addr_space="Shared")
             for w in range(W)]
    stats_i = nc.dram_tensor("stats_i", [64, 2], fp32)
    stats_o = nc.dram_tensor("stats_o", [64, 2], fp32, addr_space="Shared")
    oslice_d = nc.dram_tensor("oslice_d", [GPC, D], fp32)
    ofull_d = nc.dram_tensor("ofull_d", [G, D], fp32, addr_space="Shared")

    RG = [list(range(NC))]
    INVN = 1.0 / float(N)
    TQ = [0, 25, 50, 75, 98]          # tile ranges per quarter

    with tile.TileContext(nc) as tc:
        with (
            tc.tile_pool(name="cp", bufs=1) as cp,
            tc.tile_pool(name="hp", bufs=2) as hp,
            tc.tile_pool(name="sm", bufs=4) as sm,
            tc.tile_pool(name="slb", bufs=4) as slb,
            tc.tile_pool(name="rp", bufs=3) as rp,
            tc.tile_pool(name="ixp", bufs=4) as ixp,
            tc.tile_pool(name="ps", bufs=2, space="PSUM") as ps,
        ):
            nc.gpsimd.load_library(mlp)
            ident = cp.tile([128, 128], fp32, tag="ident")
            make_identity(nc, ident[:])

            deg_sb = cp.tile([128, NT], fp32, tag="deg")
            nc.sync.dma_start(out=deg_sb[:], in_=deg_in[:])
            dinv = cp.tile([128, NT], fp32, tag="dinv")
            nc.vector.reciprocal(out=dinv[:], in_=deg_sb[:])
            nc.scalar.activation(out=dinv[:], in_=dinv[:],
                                 func=mybir.ActivationFunctionType.Sqrt)

            w_sb = []
            bn_sb = []

            def emit_body():
                h_cur = hp.tile([128, HNPC], fp32, tag="h")
                nc.sync.dma_start(out=h_cur[:], in_=x_in[:])
                for l in range(3):
                    wt = cp.tile([128, 64], fp32, tag=f"w{l}")
                    nc.sync.dma_start(out=wt[0:64, :], in_=w_in[l * 64:(l + 1) * 64, :])
                    nc.sync.dma_start(out=wt[64:128, :], in_=w_in[l * 64:(l + 1) * 64, :])
                    w_sb.append(wt)
                    bt = cp.tile([64, 2], fp32, tag=f"bn{l}")
                    nc.sync.dma_start(out=bt[:, 0:1], in_=bn_in[(2 * l) * 64:(2 * l + 1) * 64, :])
                    nc.sync.dma_start(out=bt[:, 1:2], in_=bn_in[(2 * l + 1) * 64:(2 * l + 2) * 64, :])
                    bn_sb.append(bt)

                hhat = cp.tile([128, NT * 64], fp32, tag="hhat")
                yst = cp.tile([128, HNPC], fp32, tag="yst")

                idxp_sb = cp.tile([128, pw_off[-1]], i16, tag="idxp")
                nc.sync.dma_start(out=idxp_sb[:], in_=idxp_in[:])
                hasw_sb = cp.tile([128, W], fp32, tag="hasw")
                nc.sync.dma_start(out=hasw_sb[:], in_=hasw_in[:])
                hofs_sb = cp.tile([128, W], fp32, tag="hofs")
                nc.sync.dma_start(out=hofs_sb[:], in_=hofs_in[:])
                npad_sb = cp.tile([128, W], fp32, tag="npad")
                nc.sync.dma_start(out=npad_sb[:], in_=npad_in[:])
                anym_sb = cp.tile([128, 1], fp32, tag="anym")
                nc.sync.dma_start(out=anym_sb[:], in_=anym_in[:])
                cinv_sb = cp.tile([64, GPC], fp32, tag="cinv")
                nc.sync.dma_start(out=cinv_sb[:], in_=cntinv_in[:])
                phm_sb = cp.tile([128, 4], fp32, tag="phm")
                nc.sync.dma_start(out=phm_sb[:], in_=phm_in[:])

                def emit_gemm_layer(l, src_h):
                    # h layout: rows 0:64 = features for nodes 0..6271 (cols),
                    # rows 64:128 = features for nodes 6272..12543.
                    for t in range(NT):
                        half, c = divmod(t, 49)
                        pb = 64 * half
                        cols = slice(c * 128, (c + 1) * 128)
                        pt = ps.tile([128, 64], fp32, tag="pt", space="PSUM")
                        nc.tensor.matmul(
                            out=pt[:], lhsT=src_h[pb:pb + 64, cols],
                            rhs=w_sb[l][pb:pb + 64, :], start=True, stop=True,
                        )
                        nc.scalar.activation(
                            out=hhat[:, t * 64:(t + 1) * 64], in_=pt[:],
                            func=mybir.ActivationFunctionType.Copy,
                            scale=dinv[:, t:t + 1])

                def stage_and_gather_tables(extra_mask=False):
                    if extra_mask:
                        for qi, t in enumerate(_PHM_TILES):
                            nc.vector.tensor_scalar_mul(
                                out=hhat[:, t * 64:(t + 1) * 64],
                                in0=hhat[:, t * 64:(t + 1) * 64],
                                scalar1=phm_sb[:, qi:qi + 1])
                    for w in range(W):
                        sl_v = slice_q[w][:].rearrange("(t p) d -> p t d", p=128)
                        nc.sync.dma_start(
                            out=sl_v,
                            in_=hhat[:, TQ[w] * 64:TQ[w + 1] * 64])
                        nc.gpsimd.collective_compute(
                            "AllGather", mybir.AluOpType.bypass, replica_groups=RG,
                            ins=[slice_q[w][:].opt()], outs=[win_d[w][:].opt()],
                        )

                for l in range(3):
                    emit_gemm_layer(l, h_cur)
                    stage_and_gather_tables()

                    # ---- batched gathers + reduce -> yst ----
                    for gg in ginfo:
                        t0, Gg, Kw, SKg = gg["t0"], gg["G"], gg["Kw"], gg["SKg"]
                        idxt = ixp.tile([128, GCOLS_MAX], i16, tag="idxg")
                        nc.sync.dma_start(
                            out=idxt[:, 0:gg["gcols"]],
                            in_=idxg_in[:, gg["goff"]:gg["goff"] + gg["gcols"]])
                        slab = slb.tile([128, CAP * 64], fp32, tag="slab")
                        for w in range(W):
                            if Kw[w] == 0:
                                continue
                            NIw = 128 * Gg * Kw[w]
                            ob = gg["wbase"][w] * 64
                            outv = slab[:, ob:ob + Gg * Kw[w] * 64].rearrange(
                                "p (j d) -> p j d", d=64)
                            nc.gpsimd.dma_gather(
                                outv,
                                win_d[w][:],
                                idxt[:, gg["woff"][w]:gg["woff"][w] + 8 * Gg * Kw[w]],
                                NIw, NIw, 64, single_packet=False, queue_num=w,
                            )
                        rws = rp.tile([128, W * Gg * 64], fp32, tag="rws")
                        nw = 0
                        for w in range(W):
                            if Kw[w] == 0:
                                continue
                            ob = gg["wbase"][w] * 64
                            dstv = rws[:, nw * Gg * 64:(nw + 1) * Gg * 64]
                            if Kw[w] == 1:
                                nc.vector.tensor_copy(out=dstv, in_=slab[:, ob:ob + Gg * 64])
                            else:
                                rin = slab[:, ob:ob + Gg * Kw[w] * 64].rearrange(
                                    "p (t k d) -> p t d k", t=Gg, k=Kw[w])
                                nc.vector.reduce_sum(out=dstv, in_=rin,
                                                     axis=mybir.AxisListType.X)
                            nw += 1
                        msum = rp.tile([128, Gg * 64], fp32, tag="msum")
                        if nw == 1:
                            nc.vector.tensor_tensor(
                                out=msum[:], in0=rws[:, 0:Gg * 64],
                                in1=hhat[:, t0 * 64:(t0 + Gg) * 64],
                                op=mybir.AluOpType.add)
                        else:
                            cin = rws[:, 0:nw * Gg * 64].rearrange(
                                "p (w x) -> p x w", w=nw)
                            nc.vector.reduce_sum(out=msum[:], in_=cin,
                                                 axis=mybir.AxisListType.X)
                            nc.vector.tensor_tensor(
                                out=msum[:], in0=msum[:],
                                in1=hhat[:, t0 * 64:(t0 + Gg) * 64],
                                op=mybir.AluOpType.add)
                        msum3 = msum[:, 0:Gg * 64].rearrange("p (t d) -> p t d", d=64)
                        db = dinv[:, t0:t0 + Gg][:, :, None].broadcast_to(
                            [128, Gg, 64])
                        nc.vector.tensor_tensor(out=msum3, in0=msum3, in1=db,
                                                op=mybir.AluOpType.mult)
                        for tloc in range(Gg):
                            t = t0 + tloc
                            half, c = divmod(t, 49)
                            pb = 64 * half
                            pyt = ps.tile([64, 128], fp32, tag="pyt", space="PSUM")
                            nc.tensor.transpose(
                                out=pyt[:], in_=msum[:, tloc * 64:(tloc + 1) * 64],
                                identity=ident[:, 0:128])
                            nc.scalar.activation(
                                out=yst[pb:pb + 64, c * 128:(c + 1) * 128], in_=pyt[:],
                                func=mybir.ActivationFunctionType.Copy)

                    # ---- BN stats (sum, sumsq) over both halves ----
                    stt = sm.tile([128, 2], fp32, tag="stt")
                    sqs = cp.tile([64, 512], fp32, tag="sqs")
                    parts2 = []
                    for half in range(2):
                        pb = 64 * half
                        srow = sm.tile([128, 1], fp32, tag=f"srow{half}")
                        nc.vector.reduce_sum(out=srow[0:64, :], in_=yst[pb:pb + 64, :],
                                             axis=mybir.AxisListType.X)
                        qacc = sm.tile([128, 16], fp32, tag=f"qacc{half}")
                        nchunk = (HNPC + 511) // 512
                        for j in range(nchunk):
                            lo = j * 512
                            hi = min(lo + 512, HNPC)
                            nc.vector.tensor_tensor(
                                out=sqs[:, 0:hi - lo], in0=yst[pb:pb + 64, lo:hi],
                                in1=yst[pb:pb + 64, lo:hi], op=mybir.AluOpType.mult)
                            nc.vector.reduce_sum(
                                out=qacc[0:64, j:j + 1], in_=sqs[:, 0:hi - lo],
                                axis=mybir.AxisListType.X)
                        qsum = sm.tile([128, 1], fp32, tag=f"qsum{half}")
                        nc.vector.reduce_sum(out=qsum[0:64, :], in_=qacc[0:64, 0:nchunk],
                                             axis=mybir.AxisListType.X)
                        parts2.append((srow, qsum))
                    nc.vector.tensor_tensor(out=stt[0:64, 0:1], in0=parts2[0][0][0:64, :],
                                            in1=parts2[1][0][0:64, :], op=mybir.AluOpType.add)
                    nc.vector.tensor_tensor(out=stt[0:64, 1:2], in0=parts2[0][1][0:64, :],
                                            in1=parts2[1][1][0:64, :], op=mybir.AluOpType.add)
                    nc.sync.dma_start(out=stats_i[:], in_=stt[0:64, :])
                    nc.gpsimd.collective_compute(
                        "AllReduce", mybir.AluOpType.add, replica_groups=RG,
                        ins=[stats_i[:].opt()], outs=[stats_o[:].opt()],
                    )
                    stin = sm.tile([64, 2], fp32, tag="stin")
                    nc.sync.dma_start(out=stin[:], in_=stats_o[:])

                    # ---- BN coefficients ----
                    co = sm.tile([64, 8], fp32, tag="co")
                    mean, ex2, m2, var, rec, rstd = (co[:, i:i + 1] for i in range(6))
                    nc.vector.tensor_scalar_mul(out=mean, in0=stin[:, 0:1], scalar1=INVN)
                    nc.vector.tensor_scalar_mul(out=ex2, in0=stin[:, 1:2], scalar1=INVN)
                    nc.vector.tensor_tensor(out=m2, in0=mean, in1=mean, op=mybir.AluOpType.mult)
                    nc.vector.tensor_tensor(out=var, in0=ex2, in1=m2, op=mybir.AluOpType.subtract)
                    nc.vector.tensor_scalar_add(out=var, in0=var, scalar1=float(EPS))
                    nc.vector.reciprocal(out=rec, in_=var)
                    nc.scalar.activation(out=rstd, in_=rec, func=mybir.ActivationFunctionType.Sqrt)
                    scsh = sm.tile([128, 2], fp32, tag="scsh")
                    nc.vector.tensor_tensor(out=scsh[0:64, 0:1], in0=bn_sb[l][:, 0:1],
                                            in1=rstd, op=mybir.AluOpType.mult)
                    ms = co[:, 6:7]
                    nc.vector.tensor_tensor(out=ms, in0=mean, in1=scsh[0:64, 0:1],
                                            op=mybir.AluOpType.mult)
                    nc.vector.tensor_tensor(out=scsh[0:64, 1:2], in0=bn_sb[l][:, 1:2],
                                            in1=ms, op=mybir.AluOpType.subtract)
                    nc.vector.tensor_copy(out=scsh[64:128, :], in_=scsh[0:64, :])

                    # ---- BN apply (+ReLU) -> next h ----
                    h_nxt = hp.tile([128, HNPC], fp32, tag="h")
                    for half in range(2):
                        pb = 64 * half
                        if l < 2:
                            nc.scalar.activation(
                                out=h_nxt[pb:pb + 64, :], in_=yst[pb:pb + 64, :],
                                func=mybir.ActivationFunctionType.Relu,
                                bias=scsh[pb:pb + 64, 1:2], scale=scsh[pb:pb + 64, 0:1])
                        else:
                            nc.vector.tensor_scalar(
                                out=h_nxt[pb:pb + 64, :], in0=yst[pb:pb + 64, :],
                                scalar1=scsh[pb:pb + 64, 0:1], scalar2=scsh[pb:pb + 64, 1:2],
                                op0=mybir.AluOpType.mult, op1=mybir.AluOpType.add)
                    h_cur = h_nxt

                # ---- h3 -> table (node-major transposes + zero masks) ----
                for t in range(NT):
                    half, c = divmod(t, 49)
                    pb = 64 * half
                    ph = ps.tile([128, 64], fp32, tag="ptr", space="PSUM")
                    nc.tensor.transpose(out=ph[:], in_=h_cur[pb:pb + 64, c * 128:(c + 1) * 128],
                                        identity=ident[pb:pb + 64, pb:pb + 64])
                    nc.scalar.activation(out=hhat[:, t * 64:(t + 1) * 64], in_=ph[:],
                                         func=mybir.ActivationFunctionType.Copy)
                stage_and_gather_tables(extra_mask=True)

                # ---- pooling: per-window batched gathers ----
                ssum = sm.tile([128, 64], fp32, tag="ssum")
                smax = sm.tile([128, 64], fp32, tag="smax")
                for w in range(W):
                    kw = KPW[w]
                    nip = 128 * kw
                    slabp = slb.tile([128, CAP * 64], fp32, tag="slab")
                    assert kw <= CAP
                    outv = slabp[:, 0:kw * 64].rearrange("p (j d) -> p j d", d=64)
                    nc.gpsimd.dma_gather(
                        outv, win_d[w][:],
                        idxp_sb[:, pw_off[w]:pw_off[w + 1]],
                        nip, nip, 64, single_packet=False, queue_num=w,
                    )
                    pv = slabp[:, 0:kw * 64].rearrange("p (k d) -> p d k", k=kw)
                    ts_ = sm.tile([128, 64], fp32, tag="tsum")
                    tm_ = sm.tile([128, 64], fp32, tag="tmax")
                    if kw == 1:
                        nc.vector.tensor_copy(out=ts_[:], in_=slabp[:, 0:64])
                        nc.vector.tensor_copy(out=tm_[:], in_=slabp[:, 0:64])
                    else:
                        nc.vector.reduce_sum(out=ts_[:], in_=pv,
                                             axis=mybir.AxisListType.X)
                        nc.vector.reduce_max(out=tm_[:], in_=pv,
                                             axis=mybir.AxisListType.X)
                    # sum correction: pads duplicated first member npad_w times
                    corr = sm.tile([128, 64], fp32, tag="corr")
                    nc.vector.tensor_scalar_mul(out=corr[:], in0=slabp[:, 0:64],
                                                scalar1=npad_sb[:, w:w + 1])
                    nc.vector.tensor_tensor(out=ts_[:], in0=ts_[:], in1=corr[:],
                                            op=mybir.AluOpType.subtract)
                    # mask missing windows: tm*has + (1-has)*(-1e30)
                    nc.vector.tensor_scalar(
                        out=tm_[:], in0=tm_[:],
                        scalar1=hasw_sb[:, w:w + 1], scalar2=hofs_sb[:, w:w + 1],
                        op0=mybir.AluOpType.mult, op1=mybir.AluOpType.add)
                    if w == 0:
                        nc.vector.tensor_copy(out=ssum[:], in_=ts_[:])
                        nc.vector.tensor_copy(out=smax[:], in_=tm_[:])
                    else:
                        nc.vector.tensor_tensor(out=ssum[:], in0=ssum[:], in1=ts_[:],
                                                op=mybir.AluOpType.add)
                        nc.vector.tensor_tensor(out=smax[:], in0=smax[:], in1=tm_[:],
                                                op=mybir.AluOpType.max)
                nc.vector.tensor_scalar_mul(out=smax[:], in0=smax[:],
                                            scalar1=anym_sb[:, 0:1])

                def to_fm(srct, tg):
                    p = ps.tile([64, 128], fp32, tag="pyt", space="PSUM")
                    nc.tensor.transpose(out=p[:], in_=srct[:], identity=ident[:, 0:128])
                    tt = sm.tile([64, 128], fp32, tag="fm" + tg)
                    nc.vector.tensor_copy(out=tt[:], in_=p[:])
                    return tt

                sfm = to_fm(ssum, "s")
                mfm = to_fm(smax, "m")

                def qcombine(tsrc, op, tg):
                    v = tsrc[:].rearrange("f (g q) -> f q g", q=4)
                    a = sm.tile([64, GPC], fp32, tag="qa" + tg)
                    b = sm.tile([64, GPC], fp32, tag="qb" + tg)
                    nc.vector.tensor_tensor(out=a[:], in0=v[:, 0, :], in1=v[:, 1, :], op=op)
                    nc.vector.tensor_tensor(out=b[:], in0=v[:, 2, :], in1=v[:, 3, :], op=op)
                    nc.vector.tensor_tensor(out=a[:], in0=a[:], in1=b[:], op=op)
                    return a

                s32 = qcombine(sfm, mybir.AluOpType.add, "s")
                m32 = qcombine(mfm, mybir.AluOpType.max, "m")
                outfm = sm.tile([64, GPC], fp32, tag="outfm")
                nc.vector.tensor_tensor(out=outfm[:], in0=s32[:], in1=cinv_sb[:],
                                        op=mybir.AluOpType.mult)
                nc.vector.tensor_tensor(out=outfm[:], in0=outfm[:], in1=m32[:],
                                        op=mybir.AluOpType.add)
                po = ps.tile([GPC, 64], fp32, tag="ptr", space="PSUM")
                nc.tensor.transpose(out=po[:], in_=outfm[:], identity=ident[0:64, 0:64])
                onm = sm.tile([GPC, 64], fp32, tag="onm")
                nc.vector.tensor_copy(out=onm[:], in_=po[:])
                nc.sync.dma_start(out=oslice_d[:], in_=onm[:])
                nc.gpsimd.collective_compute(
                    "AllGather", mybir.AluOpType.bypass, replica_groups=RG,
                    ins=[oslice_d[:].opt()], outs=[ofull_d[:].opt()],
                )
                for half in range(2):
                    ot = sm.tile([128, 64], fp32, tag="ot")
                    nc.sync.dma_start(out=ot[:], in_=ofull_d[half * 128:(half + 1) * 128, :])
                    nc.sync.dma_start(out=out_ext[half * 128:(half + 1) * 128, :], in_=ot[:])

            emit_body()

    nc.compile()
    return nc


def _make_inmaps(x, prep, Ws, gs, bes):
    lpos = prep["lpos"]
    core = np.arange(N) // NPC_RAW
    w_np = np.concatenate(Ws, axis=0).astype(np.float32)
    bn_np = np.zeros((6 * 64, 1), dtype=np.float32)
    for l in range(3):
        bn_np[(2 * l) * 64:(2 * l + 1) * 64, 0] = gs[l]
        bn_np[(2 * l + 1) * 64:(2 * l + 2) * 64, 0] = bes[l]
    xl = np.zeros((NC, NPC, D), dtype=np.float32)
    xl[core, lpos] = x
    phm = _zero_masks()
    in_maps = []
    for c in range(NC):
        sl = xl[c]
        xs = np.zeros((128, HNPC), dtype=np.float32)
        xs[0:64, :] = sl[:HNPC].T
        xs[64:128, :] = sl[HNPC:].T
        in_maps.append({
            "x_in": xs,
            "w_in": w_np,
            "bn_in": bn_np,
            "deg_in": prep["deg_f"][c],
            "idxg_in": prep["idxg"][c],
            "idxp_in": prep["idxPg"][c],
            "hasw_in": prep["hasw"][c],
            "hofs_in": prep["hofs"][c],
            "npad_in": prep["npadw"][c],
            "anym_in": prep["any_m"][c][:, None].copy(),
            "phm_in": phm,
            "cntinv_in": prep["cntinv_fm"][c],
        })
    return in_maps


def _run_device(x, prep, Ws, gs, bes):
    from concourse.bass_utils import run_bass_kernel_spmd

    import os
    key = (prep["TOTC"], tuple(prep["KPW"]),
           tuple(tuple(g["Kw"]) for g in prep["ginfo"]))
    if key not in _DEVICE_CACHE:
        _DEVICE_CACHE[key] = _build_device(prep["ginfo"], prep["TOTC"],
                                           prep["KPW"], prep["pw_off"])
    nc = _DEVICE_CACHE[key]
    in_maps = _make_inmaps(x, prep, Ws, gs, bes)
    trace = bool(os.environ.get("GCN_TRACE"))
    kw = {}
    if trace:
        kw["trace"] = True
        td = os.environ.get("GCN_TRACE_DIR")
        if td:
            os.makedirs(td, exist_ok=True)
            kw["tmpdir"] = td
    res = run_bass_kernel_spmd(nc, in_maps, core_ids=list(range(NC)), **kw)
    global _LAST_RES
    _LAST_RES = res
    return np.asarray(res.results[0]["out"], dtype=np.float32)


# revision 5
# speedup vs baseline: 1.4482x; 1.0117x over previous
"""GCN feature extractor on 8 Trainium2 NeuronCores — v2.

Distribution: nodes block-sharded over 8 cores (12500 each, padded to 12544).
Each core's rows are split into 4 "quarters" (3200/3200/3200/2944 rows); the
gather table is organized as 4 windows, window w = concat over cores of their
quarter-w rows (<= 25600 rows, int16-addressable). A greedy balancer assigns
nodes to quarters so each destination's in-edges spread evenly over the 4
windows, minimizing slab padding. Per layer: transpose-free GEMM (nodes on
PSUM partitions) + dinv scale -> 4 quarter AllGathers (pipelined) -> batched
dma_gather per (tile-group, window) -> strided reduces + local self-term add.
BatchNorm stats via tiny AllReduce. Mean+max pooling via per-window batched
gathers of graph members; final [256,64] assembled with an AllGather.
"""

import numpy as np

N = 100000
E = 1600000
D = 64
G = 256
NC = 8
NPC_RAW = 12500
NPC = 12544          # 98 tiles of 128
NT = NPC // 128      # 98
EPS = 1e-5
GPC = G // NC        # 32 graphs per core
DEG_INF = np.float32(1e38)
W = 4
QSZ = [3200, 3200, 3200, 2944]        # rows per (core, quarter)
REAL = [3199, 3199, 3199, 2903]       # non-reserved rows per (core, quarter)
QB = [0, 3200, 6400, 9600]            # local block starts
WSZL = [8 * s for s in QSZ]           # window sizes
CAP = 48                              # max slab columns (64-elem units) per group
HNPC = NPC // 2
_PHM_TILES = [24, 49, 74, 97]         # tiles containing reserved zero rows


def _zero_masks():
    phm = np.ones((128, 4), dtype=np.float32)
    phm[127, 0] = 0.0   # q0 zero row: l=3199 -> t24 p127
    phm[127, 1] = 0.0   # q1: l=6399 -> t49 p127
    phm[127, 2] = 0.0   # q2: l=9599 -> t74 p127
    phm[87:, 3] = 0.0   # q3: l=12503..12543 -> t97 p87..127
    return phm


def _balance_quarters(src, dst, core, outdeg):
    """Assign each node a quarter in {0..3} balancing per-dst window counts."""
    order_e = np.argsort(src, kind="stable")
    dst_by_src = dst[order_e]
    ptr = np.concatenate([[0], np.cumsum(outdeg)])
    cnt = np.zeros((N, W), dtype=np.int32)
    quota = np.tile(np.array(REAL, dtype=np.int64), (NC, 1))
    node_order = np.argsort(-outdeg, kind="stable")
    q_of = np.full(N, -1, dtype=np.int8)
    for u in node_order:
        ds = dst_by_src[ptr[u]:ptr[u + 1]]
        c = core[u]
        if len(ds) == 0:
            q = int(np.argmax(quota[c]))
        else:
            cv = cnt[ds]
            score = (2 * cv.astype(np.int64) + 1).sum(axis=0)
            score = score + (quota[c] <= 0) * (1 << 40)
            q = int(np.argmin(score))
            cnt[ds, q] += 1
        q_of[u] = q
        quota[c, q] -= 1
    return q_of, cnt


def _pack_cols(dst_part, wvals, ranks, vals, Kw_c, woff_c, goff_c, idx_arr):
    """Shared idx packing: slot (j*128+p) -> col = goff+woff+pos//16,
    part rows pos%16 + 16r."""
    j = ranks
    pos = j * 128 + dst_part
    col = goff_c + woff_c + pos // 16
    part = (pos % 16).astype(np.int64)
    for r in range(8):
        idx_arr[part + 16 * r, col] = vals


def _preprocess(edge_index, batch):
    src = np.asarray(edge_index[0], dtype=np.int64)
    dst = np.asarray(edge_index[1], dtype=np.int64)
    batch = np.asarray(batch, dtype=np.int64)

    core = np.arange(N, dtype=np.int64) // NPC_RAW
    indeg = np.bincount(dst, minlength=N).astype(np.int64)
    outdeg = np.bincount(src, minlength=N).astype(np.int64)

    q_of, cnt = _balance_quarters(src, dst, core, outdeg)

    # local position: within (core, quarter), sort by (-max cnt, vector, -indeg)
    lpos = np.empty(N, dtype=np.int64)    # local row l in [0, NPC)
    for c in range(NC):
        for q in range(W):
            sel = np.where((core == c) & (q_of == q))[0]
            cv = cnt[sel]
            o = np.lexsort((sel, -cv[:, 3], -cv[:, 2], -cv[:, 1], -cv[:, 0],
                            -cv.max(axis=1)))
            lpos[sel[o]] = QB[q] + np.arange(len(sel))

    qq = q_of.astype(np.int64)
    kk = lpos - np.array(QB)[qq]
    widx = core * np.array(QSZ)[qq] + kk          # window-local index (int16)
    WBq = np.concatenate([[0], np.cumsum(WSZL)])
    tid = WBq[qq] + widx                          # global table row

    # per-edge: destination (tile, part), window = quarter(src)
    dst_l = lpos[dst]
    t_e = dst_l // 128
    p_e = dst_l % 128
    c_e = core[dst]
    w_e = qq[src]

    # ranks within (dst, window)
    key = ((c_e * NPC + dst_l) * W + w_e)
    eorder = np.lexsort((src, key))
    ks = key[eorder]
    seg_start = np.searchsorted(ks, np.arange(NC * NPC * W))
    rank = np.arange(E, dtype=np.int64) - seg_start[ks]

    # per (core, tile, window) K = max count
    cw_row = np.zeros((NC * NPC, W), dtype=np.int64)
    np.add.at(cw_row, (c_e * NPC + dst_l, w_e), 1)
    KT = np.zeros((NC, NT, W), dtype=np.int64)
    rows_l = np.arange(NC * NPC)
    for w in range(W):
        np.maximum.at(KT[:, :, w].reshape(-1),
                      (rows_l // NPC) * NT + (rows_l % NPC) // 128, cw_row[:, w])

    # greedy tile grouping (same structure for all cores: use per-core maxes? no,
    # groups must be identical across cores for SPMD -> take max over cores)
    KTm = KT.max(axis=0)                          # [NT, W]
    groups = []
    t0 = 0
    while t0 < NT:
        t1 = t0 + 1
        Kg = KTm[t0].copy()
        while t1 < NT:
            cand = np.maximum(Kg, KTm[t1])
            if (t1 + 1 - t0) * int(cand.sum()) > CAP:
                break
            Kg = cand
            t1 += 1
        assert (t1 - t0) * int(Kg.sum()) <= CAP
        groups.append((t0, t1, Kg.astype(np.int64)))
        t0 = t1

    ginfo = []
    goff = 0
    group_of_tile = np.zeros(NT, dtype=np.int64)
    for gi, (a, b, kw) in enumerate(groups):
        group_of_tile[a:b] = gi
        gg = b - a
        skg = int(kw.sum())
        wbase = np.concatenate([[0], np.cumsum(kw)]) * gg
        woff = 8 * gg * np.concatenate([[0], np.cumsum(kw)])
        gcols = 8 * gg * skg
        ginfo.append(dict(t0=a, G=gg, Kw=[int(x) for x in kw], SKg=skg,
                          wbase=[int(x) for x in wbase[:-1]],
                          woff=[int(x) for x in woff[:-1]],
                          goff=goff, gcols=gcols))
        goff += gcols
    TOTC = goff

    # zero-row (per core, per window) window-local index for padding slots
    zidx = np.zeros((NC, W), dtype=np.int64)
    for c in range(NC):
        for w in range(W):
            zidx[c, w] = c * QSZ[w] + REAL[w]

    t0_of = np.array([ginfo[group_of_tile[t]]["t0"] for t in range(NT)])
    KwG = np.zeros((NT, W), dtype=np.int64)
    woff_t = np.zeros((NT, W), dtype=np.int64)
    goff_t = np.zeros(NT, dtype=np.int64)
    for t in range(NT):
        gi = group_of_tile[t]
        KwG[t] = ginfo[gi]["Kw"]
        woff_t[t] = ginfo[gi]["woff"]
        goff_t[t] = ginfo[gi]["goff"]

    idxg = np.empty((NC, 128, TOTC), dtype=np.int16)
    for c in range(NC):
        for w in range(W):
            # fill pad default per (c, w): columns of window w across groups
            for gg in ginfo:
                if gg["Kw"][w] == 0:
                    continue
                lo = gg["goff"] + gg["woff"][w]
                hi = lo + 8 * gg["G"] * gg["Kw"][w]
                idxg[c, :, lo:hi] = np.int16(zidx[c, w])
    # real edges
    tloc = t_e - t0_of[t_e]
    rank_orig = np.empty(E, dtype=np.int64)
    rank_orig[eorder] = rank
    jslot = tloc * KwG[t_e, w_e] + rank_orig
    pos = jslot * 128 + p_e
    col = goff_t[t_e] + woff_t[t_e, w_e] + pos // 16
    part = (pos % 16).astype(np.int64)
    val = widx[src].astype(np.int16)
    for r in range(8):
        idxg[c_e, part + 16 * r, col] = val

    # degree (with self loop); reserved rows get INF so dinv ~ 0
    deg_f = np.full((NC, NPC), DEG_INF, dtype=np.float32)
    deg_f[core, lpos] = (indeg + 1).astype(np.float32)
    deg_f = deg_f.reshape(NC, NT, 128).transpose(0, 2, 1).copy()   # [NC,128,NT]

    # ---- pooling: per-window batched gathers ----
    cnt_g = np.bincount(batch, minlength=G).astype(np.int64)
    gstart = np.concatenate([[0], np.cumsum(cnt_g)])
    # members per (core, p=4*gl+q_lane, window)
    memb = [[[[] for _ in range(W)] for _ in range(128)] for _ in range(NC)]
    for g in range(G):
        c, gl = g // GPC, g % GPC
        mem = np.arange(gstart[g], gstart[g + 1])
        for qlane in range(4):
            p = 4 * gl + qlane
            for u in mem[qlane::4]:
                w = qq[u]
                memb[c][p][w].append(widx[u])
    KPW = np.zeros(W, dtype=np.int64)
    for w in range(W):
        KPW[w] = max(1, max(len(memb[c][p][w]) for c in range(NC)
                            for p in range(128)))
    assert max(KPW) <= CAP, KPW
    idxP = []
    hasw = np.zeros((NC, 128, W), dtype=np.float32)
    npadw = np.zeros((NC, 128, W), dtype=np.float32)
    for w in range(W):
        a = np.empty((NC, 128, int(KPW[w])), dtype=np.int16)
        for c in range(NC):
            a[c, :, :] = np.int16(zidx[c, w])
            for p in range(128):
                m = memb[c][p][w]
                if m:
                    a[c, p, :] = np.int16(m[0])   # pads = first member (max-safe)
                    a[c, p, :len(m)] = np.asarray(m, dtype=np.int16)
                    hasw[c, p, w] = 1.0
                    npadw[c, p, w] = float(KPW[w] - len(m))
        idxP.append(a)
    hofs = ((1.0 - hasw) * np.float32(-1e30)).astype(np.float32)
    # pack pooling indices in gather layout: [NC, 128, sum_w 8*KPW]
    pw_off = np.concatenate([[0], np.cumsum(8 * KPW)])
    idxPg = np.empty((NC, 128, int(pw_off[-1])), dtype=np.int16)
    for w in range(W):
        kw = int(KPW[w])
        for c in range(NC):
            flat = idxP[w][c].T.reshape(-1)        # [kw*128] in (j, p) order
            buf = flat.reshape(kw * 8, 16).T       # [16, kw*8]
            idxPg[c, :, pw_off[w]:pw_off[w + 1]] = np.tile(buf, (8, 1))
    any_m = hasw.max(axis=2)                       # [NC, 128]
    cntinv = np.zeros((NC, GPC), dtype=np.float32)
    for g in range(G):
        c, gl = g // GPC, g % GPC
        if cnt_g[g] > 0:
            cntinv[c, gl] = 1.0 / cnt_g[g]
    cntinv_fm = np.repeat(cntinv[:, None, :], D, axis=1).copy()

    return dict(
        tid=tid, lpos=lpos, deg_f=deg_f, ginfo=ginfo, idxg=idxg, TOTC=TOTC,
        idxPg=idxPg, KPW=[int(x) for x in KPW],
        pw_off=[int(x) for x in pw_off], hasw=hasw, hofs=hofs, npadw=npadw,
        any_m=any_m, cntinv_fm=cntinv_fm, qq=qq, widx=widx,
    )


def _numpy_model(x, prep, Ws, gs, bes):
    """float32 mirror of the device algorithm (validation only)."""
    lpos = prep["lpos"]
    deg = prep["deg_f"]                                    # [NC,128,NT]
    dinv = np.sqrt(np.float32(1.0) / deg).astype(np.float32)
    ginfo, idxg = prep["ginfo"], prep["idxg"]
    core = np.arange(N) // NPC_RAW

    # local x: [NC, 64, NPC] feature-major
    h_fm = np.zeros((NC, D, NPC), dtype=np.float32)
    h_fm[core, :, lpos] = x
    windows = [np.zeros((WSZL[w], D), dtype=np.float32) for w in range(W)]

    def stage_tables(hh_nm):
        # hh_nm: [NC, NPC, D] node-major table slices (dinv-scaled)
        for w in range(W):
            for c in range(NC):
                windows[w][c * QSZ[w]:(c + 1) * QSZ[w]] = (
                    hh_nm[c][QB[w]:QB[w] + QSZ[w]])

    for l in range(3):
        Wm, g_, be = Ws[l], gs[l], bes[l]
        hh_nm = np.zeros((NC, NPC, D), dtype=np.float32)
        hhat_s = np.zeros((NC, NPC, D), dtype=np.float32)
        for c in range(NC):
            t_fm = (Wm.T.astype(np.float32) @ h_fm[c]).astype(np.float32)
            dflat = dinv[c].T.reshape(-1)
            hh = (t_fm.T * dflat[:, None]).astype(np.float32)
            hh_nm[c] = hh
            hhat_s[c] = hh
        stage_tables(hh_nm)
        y_nm = np.zeros((NC, NPC, D), dtype=np.float32)
        for c in range(NC):
            for gg in ginfo:
                t0, Gg, Kw = gg["t0"], gg["G"], gg["Kw"]
                msum = np.zeros((128, Gg, D), dtype=np.float32)
                for w in range(W):
                    if Kw[w] == 0:
                        continue
                    NIw = 128 * Gg * Kw[w]
                    lo = gg["goff"] + gg["woff"][w]
                    buf = idxg[c][:16, lo:lo + 8 * Gg * Kw[w]]
                    flat = buf.T.reshape(-1)[:NIw].astype(np.int64)
                    flat = np.where(flat < 0, flat + 65536, flat)
                    ii = np.arange(NIw)
                    np.add.at(msum, (ii % 128, (ii // 128) // Kw[w]),
                              windows[w][flat])
                for tl in range(Gg):
                    t = t0 + tl
                    sl = slice(t * 128, (t + 1) * 128)
                    acc = (msum[:, tl] + hhat_s[c][sl]) * dinv[c, :, t][:, None]
                    y_nm[c][sl] = acc.astype(np.float32)
        S = y_nm.sum(axis=(0, 1), dtype=np.float32)
        Q = (y_nm.astype(np.float32) ** 2).sum(axis=(0, 1), dtype=np.float32)
        mean = (S / np.float32(N)).astype(np.float32)
        var = (Q / np.float32(N) - mean * mean).astype(np.float32)
        rstd = np.sqrt(np.float32(1.0) / (var + np.float32(EPS))).astype(np.float32)
        scale = (g_ * rstd).astype(np.float32)
        shift = (be - mean * scale).astype(np.float32)
        if l == 2:
            scale3, shift3 = scale, shift
            y3_nm = y_nm
            break
        for c in range(NC):
            h = (y_nm[c] * scale[None, :] + shift[None, :]).astype(np.float32)
            h = np.maximum(h, 0)
            h_fm[c] = h.T

    # final table: pre-BN y3 (zero rows forced to exactly 0)
    phm = _zero_masks()
    for c in range(NC):
        for qi, t in enumerate(_PHM_TILES):
            mask = phm[:, qi:qi + 1]
            y3_nm[c][t * 128:(t + 1) * 128] *= mask
    stage_tables(y3_nm)

    idxPg, KPW, pw_off = prep["idxPg"], prep["KPW"], prep["pw_off"]
    hasw, hofs, npadw = prep["hasw"], prep["hofs"], prep["npadw"]
    any_m, cntinv_fm = prep["any_m"], prep["cntinv_fm"]
    out = np.zeros((G, D), dtype=np.float32)
    for c in range(NC):
        ssum = np.zeros((128, D), dtype=np.float32)
        smax = np.full((128, D), -np.inf, dtype=np.float32)
        smin = np.full((128, D), np.inf, dtype=np.float32)
        for w in range(W):
            kw = KPW[w]
            buf = idxPg[c][:16, pw_off[w]:pw_off[w + 1]]
            flat = buf.T.reshape(-1).astype(np.int64)
            flat = np.where(flat < 0, flat + 65536, flat)
            slab = windows[w][flat].reshape(kw, 128, D).transpose(1, 0, 2)
            ts = slab.sum(axis=1, dtype=np.float32)
            ts -= npadw[c, :, w:w + 1] * slab[:, 0, :]
            ssum += ts
            pm = slab.max(axis=1)
            smax = np.maximum(smax, pm * hasw[c, :, w:w + 1] + hofs[c, :, w:w + 1])
            pn = slab.min(axis=1)
            smin = np.minimum(smin, pn * hasw[c, :, w:w + 1] - hofs[c, :, w:w + 1])
        smax = smax * any_m[c][:, None]
        smin = smin * any_m[c][:, None]
        q = ssum.reshape(GPC, 4, D).sum(axis=1, dtype=np.float32)
        m = smax.reshape(GPC, 4, D).max(axis=1)
        n = smin.reshape(GPC, 4, D).min(axis=1)
        mean_pre = q * cntinv_fm[c, 0][:, None]
        a = mean_pre * scale3[None, :] + shift3[None, :]
        b = m * scale3[None, :] + shift3[None, :]
        d = n * scale3[None, :] + shift3[None, :]
        out[c * GPC:(c + 1) * GPC] = a + np.maximum(b, d)
    return out


def kernel(**inputs):
    x = np.asarray(inputs["x"], dtype=np.float32)
    prep = _preprocess(inputs["edge_index"], inputs["batch"])
    Ws = [np.asarray(inputs[f"W{i+1}"], dtype=np.float32) for i in range(3)]
    gs = [np.asarray(inputs[f"g{i+1}"], dtype=np.float32) for i in range(3)]
    bes = [np.asarray(inputs[f"be{i+1}"], dtype=np.float32) for i in range(3)]
    import os
    if os.environ.get("GCN_NUMPY_MODEL"):
        return _numpy_model(x, prep, Ws, gs, bes)
    return _run_device(x, prep, Ws, gs, bes)


_DEVICE_CACHE = {}


def _build_device(ginfo, TOTC, KPW, pw_off):
    import concourse.bacc as bacc
    import concourse.bass as bass
    import concourse.tile as tile
    import concourse.mybir as mybir
    from concourse.masks import make_identity
    from concourse.library_config import mlp

    fp32 = mybir.dt.float32
    i16 = mybir.dt.int16
    GCOLS_MAX = max(g["gcols"] for g in ginfo)

    nc = bacc.Bacc("TRN2", target_bir_lowering=False, debug=False, num_devices=NC,
                   num_swdge_queues=4, dynamic_dma_scratch_size=32768)

    x_in = nc.dram_tensor("x_in", [128, HNPC], fp32, kind="ExternalInput")
    w_in = nc.dram_tensor("w_in", [3 * 64, 64], fp32, kind="ExternalInput")
    bn_in = nc.dram_tensor("bn_in", [6 * 64, 1], fp32, kind="ExternalInput")
    deg_in = nc.dram_tensor("deg_in", [128, NT], fp32, kind="ExternalInput")
    idxg_in = nc.dram_tensor("idxg_in", [128, TOTC], i16, kind="ExternalInput")
    idxp_in = nc.dram_tensor("idxp_in", [128, pw_off[-1]], i16, kind="ExternalInput")
    hasw_in = nc.dram_tensor("hasw_in", [128, W], fp32, kind="ExternalInput")
    hofs_in = nc.dram_tensor("hofs_in", [128, W], fp32, kind="ExternalInput")
    npad_in = nc.dram_tensor("npad_in", [128, W], fp32, kind="ExternalInput")
    anym_in = nc.dram_tensor("anym_in", [128, 1], fp32, kind="ExternalInput")
    phm_in = nc.dram_tensor("phm_in", [128, 4], fp32, kind="ExternalInput")
    cntinv_in = nc.dram_tensor("cntinv_in", [64, GPC], fp32, kind="ExternalInput")
    out_ext = nc.dram_tensor("out", [G, D], fp32, kind="ExternalOutput")

    slice_q = [nc.dram_tensor(f"slice_q{w}", [QSZ[w], D], fp32) for w in range(W)]
    win_d = [nc.dram_tensor(f"win_d{w}", [WSZL[w], D], fp32, addr_space="Shared")
             for w in range(W)]
    stats_i = nc.dram_tensor("stats_i", [64, 2], fp32)
    stats_o = nc.dram_tensor("stats_o", [64, 2], fp32, addr_space="Shared")
    oslice_d = nc.dram_tensor("oslice_d", [GPC, D], fp32)
    ofull_d = nc.dram_tensor("ofull_d", [G, D], fp32, addr_space="Shared")

    RG = [list(range(NC))]
    INVN = 1.0 / float(N)
    TQ = [0, 25, 50, 75, 98]          # tile ranges per quarter

    with tile.TileContext(nc) as tc:
        with (
            tc.tile_pool(name="cp", bufs=1) as cp,
            tc.tile_pool(name="hp", bufs=2) as hp,
            tc.tile_pool(name="sm", bufs=4) as sm,
            tc.tile_pool(name="slb", bufs=4) as slb,
            tc.tile_pool(name="rp", bufs=3) as rp,
            tc.tile_pool(name="ixp", bufs=4) as ixp,
            tc.tile_pool(name="ps", bufs=2, space="PSUM") as ps,
        ):
            nc.gpsimd.load_library(mlp)
            ident = cp.tile([128, 128], fp32, tag="ident")
            make_identity(nc, ident[:])

            deg_sb = cp.tile([128, NT], fp32, tag="deg")
            nc.sync.dma_start(out=deg_sb[:], in_=deg_in[:])
            dinv = cp.tile([128, NT], fp32, tag="dinv")
            nc.vector.reciprocal(out=dinv[:], in_=deg_sb[:])
            nc.scalar.activation(out=dinv[:], in_=dinv[:],
                                 func=mybir.ActivationFunctionType.Sqrt)

            w_sb = []
            bn_sb = []

            def emit_body():
                h_cur = hp.tile([128, HNPC], fp32, tag="h")
                nc.sync.dma_start(out=h_cur[:], in_=x_in[:])
                for l in range(3):
                    wt = cp.tile([128, 64], fp32, tag=f"w{l}")
                    nc.sync.dma_start(out=wt[0:64, :], in_=w_in[l * 64:(l + 1) * 64, :])
                    nc.sync.dma_start(out=wt[64:128, :], in_=w_in[l * 64:(l + 1) * 64, :])
                    w_sb.append(wt)
                    bt = cp.tile([64, 2], fp32, tag=f"bn{l}")
                    nc.sync.dma_start(out=bt[:, 0:1], in_=bn_in[(2 * l) * 64:(2 * l + 1) * 64, :])
                    nc.sync.dma_start(out=bt[:, 1:2], in_=bn_in[(2 * l + 1) * 64:(2 * l + 2) * 64, :])
                    bn_sb.append(bt)

                hhat = cp.tile([128, NT * 64], fp32, tag="hhat")
                yst = cp.tile([128, HNPC], fp32, tag="yst")

                idxp_sb = cp.tile([128, pw_off[-1]], i16, tag="idxp")
                nc.sync.dma_start(out=idxp_sb[:], in_=idxp_in[:])
                hasw_sb = cp.tile([128, W], fp32, tag="hasw")
                nc.sync.dma_start(out=hasw_sb[:], in_=hasw_in[:])
                hofs_sb = cp.tile([128, W], fp32, tag="hofs")
                nc.sync.dma_start(out=hofs_sb[:], in_=hofs_in[:])
                npad_sb = cp.tile([128, W], fp32, tag="npad")
                nc.sync.dma_start(out=npad_sb[:], in_=npad_in[:])
                anym_sb = cp.tile([128, 1], fp32, tag="anym")
                nc.sync.dma_start(out=anym_sb[:], in_=anym_in[:])
                cinv_sb = cp.tile([64, GPC], fp32, tag="cinv")
                nc.sync.dma_start(out=cinv_sb[:], in_=cntinv_in[:])
                phm_sb = cp.tile([128, 4], fp32, tag="phm")
                nc.sync.dma_start(out=phm_sb[:], in_=phm_in[:])

                def emit_gemm_layer(l, src_h):
                    # h layout: rows 0:64 = features for nodes 0..6271 (cols),
                    # rows 64:128 = features for nodes 6272..12543.
                    for t in range(NT):
                        half, c = divmod(t, 49)
                        pb = 64 * half
                        cols = slice(c * 128, (c + 1) * 128)
                        pt = ps.tile([128, 64], fp32, tag="pt", space="PSUM")
                        nc.tensor.matmul(
                            out=pt[:], lhsT=src_h[pb:pb + 64, cols],
                            rhs=w_sb[l][pb:pb + 64, :], start=True, stop=True,
                        )
                        nc.scalar.activation(
                            out=hhat[:, t * 64:(t + 1) * 64], in_=pt[:],
                            func=mybir.ActivationFunctionType.Copy,
                            scale=dinv[:, t:t + 1])

                def stage_and_gather_tables(extra_mask=False):
                    if extra_mask:
                        for qi, t in enumerate(_PHM_TILES):
                            nc.vector.tensor_scalar_mul(
                                out=hhat[:, t * 64:(t + 1) * 64],
                                in0=hhat[:, t * 64:(t + 1) * 64],
                                scalar1=phm_sb[:, qi:qi + 1])
                    for w in range(W):
                        sl_v = slice_q[w][:].rearrange("(t p) d -> p t d", p=128)
                        nc.sync.dma_start(
                            out=sl_v,
                            in_=hhat[:, TQ[w] * 64:TQ[w + 1] * 64])
                        nc.gpsimd.collective_compute(
                            "AllGather", mybir.AluOpType.bypass, replica_groups=RG,
                            ins=[slice_q[w][:].opt()], outs=[win_d[w][:].opt()],
                        )

                for l in range(3):
                    emit_gemm_layer(l, h_cur)
                    stage_and_gather_tables()

                    # ---- batched gathers + reduce -> yst ----
                    for gg in ginfo:
                        t0, Gg, Kw, SKg = gg["t0"], gg["G"], gg["Kw"], gg["SKg"]
                        idxt = ixp.tile([128, GCOLS_MAX], i16, tag="idxg")
                        nc.sync.dma_start(
                            out=idxt[:, 0:gg["gcols"]],
                            in_=idxg_in[:, gg["goff"]:gg["goff"] + gg["gcols"]])
                        slab = slb.tile([128, CAP * 64], fp32, tag="slab")
                        for w in range(W):
                            if Kw[w] == 0:
                                continue
                            NIw = 128 * Gg * Kw[w]
                            ob = gg["wbase"][w] * 64
                            outv = slab[:, ob:ob + Gg * Kw[w] * 64].rearrange(
                                "p (j d) -> p j d", d=64)
                            nc.gpsimd.dma_gather(
                                outv,
                                win_d[w][:],
                                idxt[:, gg["woff"][w]:gg["woff"][w] + 8 * Gg * Kw[w]],
                                NIw, NIw, 64, single_packet=False, queue_num=w,
                            )
                        rws = rp.tile([128, W * Gg * 64], fp32, tag="rws")
                        nw = 0
                        for w in range(W):
                            if Kw[w] == 0:
                                continue
                            ob = gg["wbase"][w] * 64
                            dstv = rws[:, nw * Gg * 64:(nw + 1) * Gg * 64]
                            if Kw[w] == 1:
                                nc.vector.tensor_copy(out=dstv, in_=slab[:, ob:ob + Gg * 64])
                            else:
                                rin = slab[:, ob:ob + Gg * Kw[w] * 64].rearrange(
                                    "p (t k d) -> p t d k", t=Gg, k=Kw[w])
                                nc.vector.reduce_sum(out=dstv, in_=rin,
                                                     axis=mybir.AxisListType.X)
                            nw += 1
                        msum = rp.tile([128, Gg * 64], fp32, tag="msum")
                        if nw == 1:
                            nc.vector.tensor_tensor(
                                out=msum[:], in0=rws[:, 0:Gg * 64],
                                in1=hhat[:, t0 * 64:(t0 + Gg) * 64],
                                op=mybir.AluOpType.add)
                        else:
                            cin = rws[:, 0:nw * Gg * 64].rearrange(
                                "p (w x) -> p x w", w=nw)
                            nc.vector.reduce_sum(out=msum[:], in_=cin,
                                                 axis=mybir.AxisListType.X)
                            nc.vector.tensor_tensor(
                                out=msum[:], in0=msum[:],
                                in1=hhat[:, t0 * 64:(t0 + Gg) * 64],
                                op=mybir.AluOpType.add)
                        msum3 = msum[:, 0:Gg * 64].rearrange("p (t d) -> p t d", d=64)
                        db = dinv[:, t0:t0 + Gg][:, :, None].broadcast_to(
                            [128, Gg, 64])
                        nc.vector.tensor_tensor(out=msum3, in0=msum3, in1=db,
                                                op=mybir.AluOpType.mult)
                        for tloc in range(Gg):
                            t = t0 + tloc
                            half, c = divmod(t, 49)
                            pb = 64 * half
                            pyt = ps.tile([64, 128], fp32, tag="pyt", space="PSUM")
                            nc.tensor.transpose(
                                out=pyt[:], in_=msum[:, tloc * 64:(tloc + 1) * 64],
                                identity=ident[:, 0:128])
                            nc.scalar.activation(
                                out=yst[pb:pb + 64, c * 128:(c + 1) * 128], in_=pyt[:],
                                func=mybir.ActivationFunctionType.Copy)

                    # ---- BN stats (sum, sumsq) over both halves ----
                    stt = sm.tile([128, 2], fp32, tag="stt")
                    sqs = cp.tile([64, 512], fp32, tag="sqs")
                    parts2 = []
                    for half in range(2):
                        pb = 64 * half
                        srow = sm.tile([128, 1], fp32, tag=f"srow{half}")
                        nc.vector.reduce_sum(out=srow[0:64, :], in_=yst[pb:pb + 64, :],
                                             axis=mybir.AxisListType.X)
                        qacc = sm.tile([128, 16], fp32, tag=f"qacc{half}")
                        nchunk = (HNPC + 511) // 512
                        for j in range(nchunk):
                            lo = j * 512
                            hi = min(lo + 512, HNPC)
                            nc.vector.tensor_tensor(
                                out=sqs[:, 0:hi - lo], in0=yst[pb:pb + 64, lo:hi],
                                in1=yst[pb:pb + 64, lo:hi], op=mybir.AluOpType.mult)
                            nc.vector.reduce_sum(
                                out=qacc[0:64, j:j + 1], in_=sqs[:, 0:hi - lo],
                                axis=mybir.AxisListType.X)
                        qsum = sm.tile([128, 1], fp32, tag=f"qsum{half}")
                        nc.vector.reduce_sum(out=qsum[0:64, :], in_=qacc[0:64, 0:nchunk],
                                             axis=mybir.AxisListType.X)
                        parts2.append((srow, qsum))
                    nc.vector.tensor_tensor(out=stt[0:64, 0:1], in0=parts2[0][0][0:64, :],
                                            in1=parts2[1][0][0:64, :], op=mybir.AluOpType.add)
                    nc.vector.tensor_tensor(out=stt[0:64, 1:2], in0=parts2[0][1][0:64, :],
                                            in1=parts2[1][1][0:64, :], op=mybir.AluOpType.add)
                    nc.sync.dma_start(out=stats_i[:], in_=stt[0:64, :])
                    nc.gpsimd.collective_compute(
                        "AllReduce", mybir.AluOpType.add, replica_groups=RG,
                        ins=[stats_i[:].opt()], outs=[stats_o[:].opt()],
                    )
                    if l == 2:
                        break   # BN3 folded into pooled output; stats AllReduce in flight

                    stin = sm.tile([64, 2], fp32, tag="stin")
                    nc.sync.dma_start(out=stin[:], in_=stats_o[:])

                    # ---- BN coefficients ----
                    co = sm.tile([64, 8], fp32, tag="co")
                    mean, ex2, m2, var, rec, rstd = (co[:, i:i + 1] for i in range(6))
                    nc.vector.tensor_scalar_mul(out=mean, in0=stin[:, 0:1], scalar1=INVN)
                    nc.vector.tensor_scalar_mul(out=ex2, in0=stin[:, 1:2], scalar1=INVN)
                    nc.vector.tensor_tensor(out=m2, in0=mean, in1=mean, op=mybir.AluOpType.mult)
                    nc.vector.tensor_tensor(out=var, in0=ex2, in1=m2, op=mybir.AluOpType.subtract)
                    nc.vector.tensor_scalar_add(out=var, in0=var, scalar1=float(EPS))
                    nc.vector.reciprocal(out=rec, in_=var)
                    nc.scalar.activation(out=rstd, in_=rec, func=mybir.ActivationFunctionType.Sqrt)
                    scsh = sm.tile([128, 2], fp32, tag="scsh")
                    nc.vector.tensor_tensor(out=scsh[0:64, 0:1], in0=bn_sb[l][:, 0:1],
                                            in1=rstd, op=mybir.AluOpType.mult)
                    ms = co[:, 6:7]
                    nc.vector.tensor_tensor(out=ms, in0=mean, in1=scsh[0:64, 0:1],
                                            op=mybir.AluOpType.mult)
                    nc.vector.tensor_tensor(out=scsh[0:64, 1:2], in0=bn_sb[l][:, 1:2],
                                            in1=ms, op=mybir.AluOpType.subtract)
                    nc.vector.tensor_copy(out=scsh[64:128, :], in_=scsh[0:64, :])

                    # ---- BN apply (+ReLU) -> next h ----
                    h_nxt = hp.tile([128, HNPC], fp32, tag="h")
                    for half in range(2):
                        pb = 64 * half
                        nc.scalar.activation(
                            out=h_nxt[pb:pb + 64, :], in_=yst[pb:pb + 64, :],
                            func=mybir.ActivationFunctionType.Relu,
                            bias=scsh[pb:pb + 64, 1:2], scale=scsh[pb:pb + 64, 0:1])
                    h_cur = h_nxt

                # ---- y3 (pre-BN) -> table (node-major transposes + zero masks) ----
                for t in range(NT):
                    half, c = divmod(t, 49)
                    pb = 64 * half
                    ph = ps.tile([128, 64], fp32, tag="ptr", space="PSUM")
                    nc.tensor.transpose(out=ph[:], in_=yst[pb:pb + 64, c * 128:(c + 1) * 128],
                                        identity=ident[pb:pb + 64, pb:pb + 64])
                    nc.scalar.activation(out=hhat[:, t * 64:(t + 1) * 64], in_=ph[:],
                                         func=mybir.ActivationFunctionType.Copy)
                stage_and_gather_tables(extra_mask=True)

                # ---- pooling (pre-BN): per-window batched gathers ----
                ssum = sm.tile([128, 64], fp32, tag="ssum")
                smax = sm.tile([128, 64], fp32, tag="smax")
                smin = sm.tile([128, 64], fp32, tag="smin")
                for w in range(W):
                    kw = KPW[w]
                    nip = 128 * kw
                    slabp = slb.tile([128, CAP * 64], fp32, tag="slab")
                    assert kw <= CAP
                    outv = slabp[:, 0:kw * 64].rearrange("p (j d) -> p j d", d=64)
                    nc.gpsimd.dma_gather(
                        outv, win_d[w][:],
                        idxp_sb[:, pw_off[w]:pw_off[w + 1]],
                        nip, nip, 64, single_packet=False, queue_num=w,
                    )
                    pv = slabp[:, 0:kw * 64].rearrange("p (k d) -> p d k", k=kw)
                    ts_ = sm.tile([128, 64], fp32, tag="tsum")
                    tm_ = sm.tile([128, 64], fp32, tag="tmax")
                    tn_ = sm.tile([128, 64], fp32, tag="tmin")
                    if kw == 1:
                        nc.vector.tensor_copy(out=ts_[:], in_=slabp[:, 0:64])
                        nc.vector.tensor_copy(out=tm_[:], in_=slabp[:, 0:64])
                        nc.vector.tensor_copy(out=tn_[:], in_=slabp[:, 0:64])
                    else:
                        nc.vector.reduce_sum(out=ts_[:], in_=pv,
                                             axis=mybir.AxisListType.X)
                        nc.vector.reduce_max(out=tm_[:], in_=pv,
                                             axis=mybir.AxisListType.X)
                        nc.vector.tensor_reduce(out=tn_[:], in_=pv,
                                                axis=mybir.AxisListType.X,
                                                op=mybir.AluOpType.min)
                    # sum correction: pads duplicated first member npad_w times
                    corr = sm.tile([128, 64], fp32, tag="corr")
                    nc.vector.tensor_scalar_mul(out=corr[:], in0=slabp[:, 0:64],
                                                scalar1=npad_sb[:, w:w + 1])
                    nc.vector.tensor_tensor(out=ts_[:], in0=ts_[:], in1=corr[:],
                                            op=mybir.AluOpType.subtract)
                    # mask missing windows: tm*has - 1e30*(1-has); tn*has + 1e30*(1-has)
                    nc.vector.tensor_scalar(
                        out=tm_[:], in0=tm_[:],
                        scalar1=hasw_sb[:, w:w + 1], scalar2=hofs_sb[:, w:w + 1],
                        op0=mybir.AluOpType.mult, op1=mybir.AluOpType.add)
                    nc.vector.tensor_scalar(
                        out=tn_[:], in0=tn_[:],
                        scalar1=hasw_sb[:, w:w + 1], scalar2=hofs_sb[:, w:w + 1],
                        op0=mybir.AluOpType.mult, op1=mybir.AluOpType.subtract)
                    if w == 0:
                        nc.vector.tensor_copy(out=ssum[:], in_=ts_[:])
                        nc.vector.tensor_copy(out=smax[:], in_=tm_[:])
                        nc.vector.tensor_copy(out=smin[:], in_=tn_[:])
                    else:
                        nc.vector.tensor_tensor(out=ssum[:], in0=ssum[:], in1=ts_[:],
                                                op=mybir.AluOpType.add)
                        nc.vector.tensor_tensor(out=smax[:], in0=smax[:], in1=tm_[:],
                                                op=mybir.AluOpType.max)
                        nc.vector.tensor_tensor(out=smin[:], in0=smin[:], in1=tn_[:],
                                                op=mybir.AluOpType.min)
                nc.vector.tensor_scalar_mul(out=smax[:], in0=smax[:],
                                            scalar1=anym_sb[:, 0:1])
                nc.vector.tensor_scalar_mul(out=smin[:], in0=smin[:],
                                            scalar1=anym_sb[:, 0:1])

                def to_fm(srct, tg):
                    p = ps.tile([64, 128], fp32, tag="pyt", space="PSUM")
                    nc.tensor.transpose(out=p[:], in_=srct[:], identity=ident[:, 0:128])
                    tt = sm.tile([64, 128], fp32, tag="fm" + tg)
                    nc.vector.tensor_copy(out=tt[:], in_=p[:])
                    return tt

                sfm = to_fm(ssum, "s")
                mfm = to_fm(smax, "m")
                nfm = to_fm(smin, "n")

                def qcombine(tsrc, op, tg):
                    v = tsrc[:].rearrange("f (g q) -> f q g", q=4)
                    a = sm.tile([64, GPC], fp32, tag="qa" + tg)
                    b = sm.tile([64, GPC], fp32, tag="qb" + tg)
                    nc.vector.tensor_tensor(out=a[:], in0=v[:, 0, :], in1=v[:, 1, :], op=op)
                    nc.vector.tensor_tensor(out=b[:], in0=v[:, 2, :], in1=v[:, 3, :], op=op)
                    nc.vector.tensor_tensor(out=a[:], in0=a[:], in1=b[:], op=op)
                    return a

                s32 = qcombine(sfm, mybir.AluOpType.add, "s")
                m32 = qcombine(mfm, mybir.AluOpType.max, "m")
                n32 = qcombine(nfm, mybir.AluOpType.min, "n")

                # ---- BN3 coefficients (AllReduce overlapped with pooling) ----
                stin = sm.tile([64, 2], fp32, tag="stin")
                nc.sync.dma_start(out=stin[:], in_=stats_o[:])
                co = sm.tile([64, 8], fp32, tag="co")
                mean, ex2, m2, var, rec, rstd = (co[:, i:i + 1] for i in range(6))
                nc.vector.tensor_scalar_mul(out=mean, in0=stin[:, 0:1], scalar1=INVN)
                nc.vector.tensor_scalar_mul(out=ex2, in0=stin[:, 1:2], scalar1=INVN)
                nc.vector.tensor_tensor(out=m2, in0=mean, in1=mean, op=mybir.AluOpType.mult)
                nc.vector.tensor_tensor(out=var, in0=ex2, in1=m2, op=mybir.AluOpType.subtract)
                nc.vector.tensor_scalar_add(out=var, in0=var, scalar1=float(EPS))
                nc.vector.reciprocal(out=rec, in_=var)
                nc.scalar.activation(out=rstd, in_=rec, func=mybir.ActivationFunctionType.Sqrt)
                scsh = sm.tile([128, 2], fp32, tag="scsh")
                nc.vector.tensor_tensor(out=scsh[0:64, 0:1], in0=bn_sb[2][:, 0:1],
                                        in1=rstd, op=mybir.AluOpType.mult)
                ms = co[:, 6:7]
                nc.vector.tensor_tensor(out=ms, in0=mean, in1=scsh[0:64, 0:1],
                                        op=mybir.AluOpType.mult)
                nc.vector.tensor_tensor(out=scsh[0:64, 1:2], in0=bn_sb[2][:, 1:2],
                                        in1=ms, op=mybir.AluOpType.subtract)

                # out = BN(mean) + max(BN(max), BN(min)) per feature
                outfm = sm.tile([64, GPC], fp32, tag="outfm")
                nc.vector.tensor_tensor(out=outfm[:], in0=s32[:], in1=cinv_sb[:],
                                        op=mybir.AluOpType.mult)
                nc.vector.tensor_scalar(
                    out=outfm[:], in0=outfm[:], scalar1=scsh[0:64, 0:1],
                    scalar2=scsh[0:64, 1:2],
                    op0=mybir.AluOpType.mult, op1=mybir.AluOpType.add)
                nc.vector.tensor_scalar(
                    out=m32[:], in0=m32[:], scalar1=scsh[0:64, 0:1],
                    scalar2=scsh[0:64, 1:2],
                    op0=mybir.AluOpType.mult, op1=mybir.AluOpType.add)
                nc.vector.tensor_scalar(
                    out=n32[:], in0=n32[:], scalar1=scsh[0:64, 0:1],
                    scalar2=scsh[0:64, 1:2],
                    op0=mybir.AluOpType.mult, op1=mybir.AluOpType.add)
                nc.vector.tensor_tensor(out=m32[:], in0=m32[:], in1=n32[:],
                                        op=mybir.AluOpType.max)
                nc.vector.tensor_tensor(out=outfm[:], in0=outfm[:], in1=m32[:],
                                        op=mybir.AluOpType.add)
                po = ps.tile([GPC, 64], fp32, tag="ptr", space="PSUM")
                nc.tensor.transpose(out=po[:], in_=outfm[:], identity=ident[0:64, 0:64])
                onm = sm.tile([GPC, 64], fp32, tag="onm")
                nc.vector.tensor_copy(out=onm[:], in_=po[:])
                nc.sync.dma_start(out=oslice_d[:], in_=onm[:])
                nc.gpsimd.collective_compute(
                    "AllGather", mybir.AluOpType.bypass, replica_groups=RG,
                    ins=[oslice_d[:].opt()], outs=[ofull_d[:].opt()],
                )
                for half in range(2):
                    ot = sm.tile([128, 64], fp32, tag="ot")
                    nc.sync.dma_start(out=ot[:], in_=ofull_d[half * 128:(half + 1) * 128, :])
                    nc.sync.dma_start(out=out_ext[half * 128:(half + 1) * 128, :], in_=ot[:])

            emit_body()

    nc.compile()
    return nc


def _make_inmaps(x, prep, Ws, gs, bes):
    lpos = prep["lpos"]
    core = np.arange(N) // NPC_RAW
    w_np = np.concatenate(Ws, axis=0).astype(np.float32)
    bn_np = np.zeros((6 * 64, 1), dtype=np.float32)
    for l in range(3):
        bn_np[(2 * l) * 64:(2 * l + 1) * 64, 0] = gs[l]
        bn_np[(2 * l + 1) * 64:(2 * l + 2) * 64, 0] = bes[l]
    xl = np.zeros((NC, NPC, D), dtype=np.float32)
    xl[core, lpos] = x
    phm = _zero_masks()
    in_maps = []
    for c in range(NC):
        sl = xl[c]
        xs = np.zeros((128, HNPC), dtype=np.float32)
        xs[0:64, :] = sl[:HNPC].T
        xs[64:128, :] = sl[HNPC:].T
        in_maps.append({
            "x_in": xs,
            "w_in": w_np,
            "bn_in": bn_np,
            "deg_in": prep["deg_f"][c],
            "idxg_in": prep["idxg"][c],
            "idxp_in": prep["idxPg"][c],
            "hasw_in": prep["hasw"][c],
            "hofs_in": prep["hofs"][c],
            "npad_in": prep["npadw"][c],
            "anym_in": prep["any_m"][c][:, None].copy(),
            "phm_in": phm,
            "cntinv_in": prep["cntinv_fm"][c],
        })
    return in_maps


def _run_device(x, prep, Ws, gs, bes):
    from concourse.bass_utils import run_bass_kernel_spmd

    import os
    key = (prep["TOTC"], tuple(prep["KPW"]),
           tuple(tuple(g["Kw"]) for g in prep["ginfo"]))
    if key not in _DEVICE_CACHE:
        _DEVICE_CACHE[key] = _build_device(prep["ginfo"], prep["TOTC"],
                                           prep["KPW"], prep["pw_off"])
    nc = _DEVICE_CACHE[key]
    in_maps = _make_inmaps(x, prep, Ws, gs, bes)
    trace = bool(os.environ.get("GCN_TRACE"))
    kw = {}
    if trace:
        kw["trace"] = True
        td = os.environ.get("GCN_TRACE_DIR")
        if td:
            os.makedirs(td, exist_ok=True)
            kw["tmpdir"] = td
    res = run_bass_kernel_spmd(nc, in_maps, core_ids=list(range(NC)), **kw)
    global _LAST_RES
    _LAST_RES = res
    return np.asarray(res.results[0]["out"], dtype=np.float32)


# revision 7
# speedup vs baseline: 1.5009x; 1.0364x over previous
"""GCN feature extractor on 8 Trainium2 NeuronCores — v2.

Distribution: nodes block-sharded over 8 cores (12500 each, padded to 12544).
Each core's rows are split into 4 "quarters" (3200/3200/3200/2944 rows); the
gather table is organized as 4 windows, window w = concat over cores of their
quarter-w rows (<= 25600 rows, int16-addressable). A greedy balancer assigns
nodes to quarters so each destination's in-edges spread evenly over the 4
windows, minimizing slab padding. Per layer: transpose-free GEMM (nodes on
PSUM partitions) + dinv scale -> 4 quarter AllGathers (pipelined) -> batched
dma_gather per (tile-group, window) -> strided reduces + local self-term add.
BatchNorm stats via tiny AllReduce. Mean+max pooling via per-window batched
gathers of graph members; final [256,64] assembled with an AllGather.
"""

import numpy as np

N = 100000
E = 1600000
D = 64
G = 256
NC = 8
NPC_RAW = 12500
NPC = 12544          # 98 tiles of 128
NT = NPC // 128      # 98
EPS = 1e-5
GPC = G // NC        # 32 graphs per core
DEG_INF = np.float32(1e38)
W = 4
QSZ = [3200, 3200, 3200, 2944]        # rows per (core, quarter)
REAL = [3199, 3199, 3199, 2903]       # non-reserved rows per (core, quarter)
QB = [0, 3200, 6400, 9600]            # local block starts
WSZL = [8 * s for s in QSZ]           # window sizes
CAP = 48                              # max slab columns (64-elem units) per group
HNPC = NPC // 2
_PHM_TILES = [24, 49, 74, 97]         # tiles containing reserved zero rows


def _zero_masks():
    phm = np.ones((128, 4), dtype=np.float32)
    phm[127, 0] = 0.0   # q0 zero row: l=3199 -> t24 p127
    phm[127, 1] = 0.0   # q1: l=6399 -> t49 p127
    phm[127, 2] = 0.0   # q2: l=9599 -> t74 p127
    phm[87:, 3] = 0.0   # q3: l=12503..12543 -> t97 p87..127
    return phm


def _balance_quarters(src, dst, core, outdeg):
    """Assign each node a quarter in {0..3} balancing per-dst window counts."""
    order_e = np.argsort(src, kind="stable")
    dst_by_src = dst[order_e]
    ptr = np.concatenate([[0], np.cumsum(outdeg)])
    cnt = np.zeros((N, W), dtype=np.int32)
    quota = np.tile(np.array(REAL, dtype=np.int64), (NC, 1))
    node_order = np.argsort(-outdeg, kind="stable")
    q_of = np.full(N, -1, dtype=np.int8)
    for u in node_order:
        ds = dst_by_src[ptr[u]:ptr[u + 1]]
        c = core[u]
        if len(ds) == 0:
            q = int(np.argmax(quota[c]))
        else:
            cv = cnt[ds]
            score = (2 * cv.astype(np.int64) + 1).sum(axis=0)
            score = score + (quota[c] <= 0) * (1 << 40)
            q = int(np.argmin(score))
            cnt[ds, q] += 1
        q_of[u] = q
        quota[c, q] -= 1
    return q_of, cnt


def _pack_cols(dst_part, wvals, ranks, vals, Kw_c, woff_c, goff_c, idx_arr):
    """Shared idx packing: slot (j*128+p) -> col = goff+woff+pos//16,
    part rows pos%16 + 16r."""
    j = ranks
    pos = j * 128 + dst_part
    col = goff_c + woff_c + pos // 16
    part = (pos % 16).astype(np.int64)
    for r in range(8):
        idx_arr[part + 16 * r, col] = vals


def _preprocess(edge_index, batch):
    src = np.asarray(edge_index[0], dtype=np.int64)
    dst = np.asarray(edge_index[1], dtype=np.int64)
    batch = np.asarray(batch, dtype=np.int64)

    core = np.arange(N, dtype=np.int64) // NPC_RAW
    indeg = np.bincount(dst, minlength=N).astype(np.int64)
    outdeg = np.bincount(src, minlength=N).astype(np.int64)

    q_of, cnt = _balance_quarters(src, dst, core, outdeg)

    # local position: within (core, quarter), sort by (-max cnt, vector, -indeg)
    lpos = np.empty(N, dtype=np.int64)    # local row l in [0, NPC)
    for c in range(NC):
        for q in range(W):
            sel = np.where((core == c) & (q_of == q))[0]
            cv = cnt[sel]
            o = np.lexsort((sel, -cv[:, 3], -cv[:, 2], -cv[:, 1], -cv[:, 0],
                            -cv.max(axis=1)))
            lpos[sel[o]] = QB[q] + np.arange(len(sel))

    qq = q_of.astype(np.int64)
    kk = lpos - np.array(QB)[qq]
    widx = core * np.array(QSZ)[qq] + kk          # window-local index (int16)
    WBq = np.concatenate([[0], np.cumsum(WSZL)])
    tid = WBq[qq] + widx                          # global table row

    # per-edge: destination (tile, part), window = quarter(src)
    dst_l = lpos[dst]
    t_e = dst_l // 128
    p_e = dst_l % 128
    c_e = core[dst]
    w_e = qq[src]

    # ranks within (dst, window)
    key = ((c_e * NPC + dst_l) * W + w_e)
    eorder = np.lexsort((src, key))
    ks = key[eorder]
    seg_start = np.searchsorted(ks, np.arange(NC * NPC * W))
    rank = np.arange(E, dtype=np.int64) - seg_start[ks]

    # per (core, tile, window) K = max count
    cw_row = np.zeros((NC * NPC, W), dtype=np.int64)
    np.add.at(cw_row, (c_e * NPC + dst_l, w_e), 1)
    KT = np.zeros((NC, NT, W), dtype=np.int64)
    rows_l = np.arange(NC * NPC)
    for w in range(W):
        np.maximum.at(KT[:, :, w].reshape(-1),
                      (rows_l // NPC) * NT + (rows_l % NPC) // 128, cw_row[:, w])

    # greedy tile grouping (same structure for all cores: use per-core maxes? no,
    # groups must be identical across cores for SPMD -> take max over cores)
    KTm = KT.max(axis=0)                          # [NT, W]
    groups = []
    t0 = 0
    while t0 < NT:
        t1 = t0 + 1
        Kg = KTm[t0].copy()
        while t1 < NT:
            cand = np.maximum(Kg, KTm[t1])
            if (t1 + 1 - t0) * int(cand.sum()) > CAP:
                break
            Kg = cand
            t1 += 1
        assert (t1 - t0) * int(Kg.sum()) <= CAP
        groups.append((t0, t1, Kg.astype(np.int64)))
        t0 = t1

    ginfo = []
    goff = 0
    group_of_tile = np.zeros(NT, dtype=np.int64)
    for gi, (a, b, kw) in enumerate(groups):
        group_of_tile[a:b] = gi
        gg = b - a
        skg = int(kw.sum())
        wbase = np.concatenate([[0], np.cumsum(kw)]) * gg
        woff = 8 * gg * np.concatenate([[0], np.cumsum(kw)])
        gcols = 8 * gg * skg
        ginfo.append(dict(t0=a, G=gg, Kw=[int(x) for x in kw], SKg=skg,
                          wbase=[int(x) for x in wbase[:-1]],
                          woff=[int(x) for x in woff[:-1]],
                          goff=goff, gcols=gcols))
        goff += gcols
    TOTC = goff

    # zero-row (per core, per window) window-local index for padding slots
    zidx = np.zeros((NC, W), dtype=np.int64)
    for c in range(NC):
        for w in range(W):
            zidx[c, w] = c * QSZ[w] + REAL[w]

    t0_of = np.array([ginfo[group_of_tile[t]]["t0"] for t in range(NT)])
    KwG = np.zeros((NT, W), dtype=np.int64)
    woff_t = np.zeros((NT, W), dtype=np.int64)
    goff_t = np.zeros(NT, dtype=np.int64)
    for t in range(NT):
        gi = group_of_tile[t]
        KwG[t] = ginfo[gi]["Kw"]
        woff_t[t] = ginfo[gi]["woff"]
        goff_t[t] = ginfo[gi]["goff"]

    idxg = np.empty((NC, 128, TOTC), dtype=np.int16)
    for c in range(NC):
        for w in range(W):
            # fill pad default per (c, w): columns of window w across groups
            for gg in ginfo:
                if gg["Kw"][w] == 0:
                    continue
                lo = gg["goff"] + gg["woff"][w]
                hi = lo + 8 * gg["G"] * gg["Kw"][w]
                idxg[c, :, lo:hi] = np.int16(zidx[c, w])
    # real edges
    tloc = t_e - t0_of[t_e]
    rank_orig = np.empty(E, dtype=np.int64)
    rank_orig[eorder] = rank
    jslot = tloc * KwG[t_e, w_e] + rank_orig
    pos = jslot * 128 + p_e
    col = goff_t[t_e] + woff_t[t_e, w_e] + pos // 16
    part = (pos % 16).astype(np.int64)
    val = widx[src].astype(np.int16)
    for r in range(8):
        idxg[c_e, part + 16 * r, col] = val

    # degree (with self loop); reserved rows get INF so dinv ~ 0
    deg_f = np.full((NC, NPC), DEG_INF, dtype=np.float32)
    deg_f[core, lpos] = (indeg + 1).astype(np.float32)
    deg_f = deg_f.reshape(NC, NT, 128).transpose(0, 2, 1).copy()   # [NC,128,NT]

    # ---- pooling: per-window batched gathers ----
    cnt_g = np.bincount(batch, minlength=G).astype(np.int64)
    gstart = np.concatenate([[0], np.cumsum(cnt_g)])
    # members per (core, p=4*gl+q_lane, window)
    memb = [[[[] for _ in range(W)] for _ in range(128)] for _ in range(NC)]
    for g in range(G):
        c, gl = g // GPC, g % GPC
        mem = np.arange(gstart[g], gstart[g + 1])
        for qlane in range(4):
            p = 4 * gl + qlane
            for u in mem[qlane::4]:
                w = qq[u]
                memb[c][p][w].append(widx[u])
    KPW = np.zeros(W, dtype=np.int64)
    for w in range(W):
        KPW[w] = max(1, max(len(memb[c][p][w]) for c in range(NC)
                            for p in range(128)))
    assert max(KPW) <= CAP, KPW
    idxP = []
    hasw = np.zeros((NC, 128, W), dtype=np.float32)
    npadw = np.zeros((NC, 128, W), dtype=np.float32)
    for w in range(W):
        a = np.empty((NC, 128, int(KPW[w])), dtype=np.int16)
        for c in range(NC):
            a[c, :, :] = np.int16(zidx[c, w])
            for p in range(128):
                m = memb[c][p][w]
                if m:
                    a[c, p, :] = np.int16(m[0])   # pads = first member (max-safe)
                    a[c, p, :len(m)] = np.asarray(m, dtype=np.int16)
                    hasw[c, p, w] = 1.0
                    npadw[c, p, w] = float(KPW[w] - len(m))
        idxP.append(a)
    hofs = ((1.0 - hasw) * np.float32(-1e30)).astype(np.float32)
    # pack pooling indices in gather layout: [NC, 128, sum_w 8*KPW]
    pw_off = np.concatenate([[0], np.cumsum(8 * KPW)])
    idxPg = np.empty((NC, 128, int(pw_off[-1])), dtype=np.int16)
    for w in range(W):
        kw = int(KPW[w])
        for c in range(NC):
            flat = idxP[w][c].T.reshape(-1)        # [kw*128] in (j, p) order
            buf = flat.reshape(kw * 8, 16).T       # [16, kw*8]
            idxPg[c, :, pw_off[w]:pw_off[w + 1]] = np.tile(buf, (8, 1))
    any_m = hasw.max(axis=2)                       # [NC, 128]
    cntinv = np.zeros((NC, GPC), dtype=np.float32)
    for g in range(G):
        c, gl = g // GPC, g % GPC
        if cnt_g[g] > 0:
            cntinv[c, gl] = 1.0 / cnt_g[g]
    cntinv_fm = np.repeat(cntinv[:, None, :], D, axis=1).copy()

    return dict(
        tid=tid, lpos=lpos, deg_f=deg_f, ginfo=ginfo, idxg=idxg, TOTC=TOTC,
        idxPg=idxPg, KPW=[int(x) for x in KPW],
        pw_off=[int(x) for x in pw_off], hasw=hasw, hofs=hofs, npadw=npadw,
        any_m=any_m, cntinv_fm=cntinv_fm, qq=qq, widx=widx,
    )


def _numpy_model(x, prep, Ws, gs, bes):
    """float32 mirror of the device algorithm (validation only)."""
    lpos = prep["lpos"]
    deg = prep["deg_f"]                                    # [NC,128,NT]
    dinv = np.sqrt(np.float32(1.0) / deg).astype(np.float32)
    ginfo, idxg = prep["ginfo"], prep["idxg"]
    core = np.arange(N) // NPC_RAW

    # local x: [NC, 64, NPC] feature-major
    h_fm = np.zeros((NC, D, NPC), dtype=np.float32)
    h_fm[core, :, lpos] = x
    windows = [np.zeros((WSZL[w], D), dtype=np.float32) for w in range(W)]

    def stage_tables(hh_nm):
        # hh_nm: [NC, NPC, D] node-major table slices (dinv-scaled)
        for w in range(W):
            for c in range(NC):
                windows[w][c * QSZ[w]:(c + 1) * QSZ[w]] = (
                    hh_nm[c][QB[w]:QB[w] + QSZ[w]])

    for l in range(3):
        Wm, g_, be = Ws[l], gs[l], bes[l]
        hh_nm = np.zeros((NC, NPC, D), dtype=np.float32)
        hhat_s = np.zeros((NC, NPC, D), dtype=np.float32)
        for c in range(NC):
            t_fm = (Wm.T.astype(np.float32) @ h_fm[c]).astype(np.float32)
            dflat = dinv[c].T.reshape(-1)
            hh = (t_fm.T * dflat[:, None]).astype(np.float32)
            hh_nm[c] = hh
            hhat_s[c] = hh
        stage_tables(hh_nm)
        y_nm = np.zeros((NC, NPC, D), dtype=np.float32)
        for c in range(NC):
            for gg in ginfo:
                t0, Gg, Kw = gg["t0"], gg["G"], gg["Kw"]
                msum = np.zeros((128, Gg, D), dtype=np.float32)
                for w in range(W):
                    if Kw[w] == 0:
                        continue
                    NIw = 128 * Gg * Kw[w]
                    lo = gg["goff"] + gg["woff"][w]
                    buf = idxg[c][:16, lo:lo + 8 * Gg * Kw[w]]
                    flat = buf.T.reshape(-1)[:NIw].astype(np.int64)
                    flat = np.where(flat < 0, flat + 65536, flat)
                    ii = np.arange(NIw)
                    np.add.at(msum, (ii % 128, (ii // 128) // Kw[w]),
                              windows[w][flat])
                for tl in range(Gg):
                    t = t0 + tl
                    sl = slice(t * 128, (t + 1) * 128)
                    acc = (msum[:, tl] + hhat_s[c][sl]) * dinv[c, :, t][:, None]
                    y_nm[c][sl] = acc.astype(np.float32)
        S = y_nm.sum(axis=(0, 1), dtype=np.float32)
        Q = (y_nm.astype(np.float32) ** 2).sum(axis=(0, 1), dtype=np.float32)
        mean = (S / np.float32(N)).astype(np.float32)
        var = (Q / np.float32(N) - mean * mean).astype(np.float32)
        rstd = np.sqrt(np.float32(1.0) / (var + np.float32(EPS))).astype(np.float32)
        scale = (g_ * rstd).astype(np.float32)
        shift = (be - mean * scale).astype(np.float32)
        if l == 2:
            scale3, shift3 = scale, shift
            y3_nm = y_nm
            break
        for c in range(NC):
            h = (y_nm[c] * scale[None, :] + shift[None, :]).astype(np.float32)
            h = np.maximum(h, 0)
            h_fm[c] = h.T

    # final table: pre-BN y3 (zero rows forced to exactly 0)
    phm = _zero_masks()
    for c in range(NC):
        for qi, t in enumerate(_PHM_TILES):
            mask = phm[:, qi:qi + 1]
            y3_nm[c][t * 128:(t + 1) * 128] *= mask
    stage_tables(y3_nm)

    idxPg, KPW, pw_off = prep["idxPg"], prep["KPW"], prep["pw_off"]
    hasw, hofs, npadw = prep["hasw"], prep["hofs"], prep["npadw"]
    any_m, cntinv_fm = prep["any_m"], prep["cntinv_fm"]
    out = np.zeros((G, D), dtype=np.float32)
    for c in range(NC):
        ssum = np.zeros((128, D), dtype=np.float32)
        smax = np.full((128, D), -np.inf, dtype=np.float32)
        smin = np.full((128, D), np.inf, dtype=np.float32)
        for w in range(W):
            kw = KPW[w]
            buf = idxPg[c][:16, pw_off[w]:pw_off[w + 1]]
            flat = buf.T.reshape(-1).astype(np.int64)
            flat = np.where(flat < 0, flat + 65536, flat)
            slab = windows[w][flat].reshape(kw, 128, D).transpose(1, 0, 2)
            ts = slab.sum(axis=1, dtype=np.float32)
            ts -= npadw[c, :, w:w + 1] * slab[:, 0, :]
            ssum += ts
            pm = slab.max(axis=1)
            smax = np.maximum(smax, pm * hasw[c, :, w:w + 1] + hofs[c, :, w:w + 1])
            pn = slab.min(axis=1)
            smin = np.minimum(smin, pn * hasw[c, :, w:w + 1] - hofs[c, :, w:w + 1])
        smax = smax * any_m[c][:, None]
        smin = smin * any_m[c][:, None]
        q = ssum.reshape(GPC, 4, D).sum(axis=1, dtype=np.float32)
        m = smax.reshape(GPC, 4, D).max(axis=1)
        n = smin.reshape(GPC, 4, D).min(axis=1)
        mean_pre = q * cntinv_fm[c, 0][:, None]
        a = mean_pre * scale3[None, :] + shift3[None, :]
        b = m * scale3[None, :] + shift3[None, :]
        d = n * scale3[None, :] + shift3[None, :]
        out[c * GPC:(c + 1) * GPC] = a + np.maximum(b, d)
    return out


def kernel(**inputs):
    x = np.asarray(inputs["x"], dtype=np.float32)
    prep = _preprocess(inputs["edge_index"], inputs["batch"])
    Ws = [np.asarray(inputs[f"W{i+1}"], dtype=np.float32) for i in range(3)]
    gs = [np.asarray(inputs[f"g{i+1}"], dtype=np.float32) for i in range(3)]
    bes = [np.asarray(inputs[f"be{i+1}"], dtype=np.float32) for i in range(3)]
    import os
    if os.environ.get("GCN_NUMPY_MODEL"):
        return _numpy_model(x, prep, Ws, gs, bes)
    return _run_device(x, prep, Ws, gs, bes)


_DEVICE_CACHE = {}


def _build_device(ginfo, TOTC, KPW, pw_off):
    import concourse.bacc as bacc
    import concourse.bass as bass
    import concourse.tile as tile
    import concourse.mybir as mybir
    from concourse.masks import make_identity
    from concourse.library_config import mlp

    fp32 = mybir.dt.float32
    i16 = mybir.dt.int16
    GCOLS_MAX = max(g["gcols"] for g in ginfo)

    nc = bacc.Bacc("TRN2", target_bir_lowering=False, debug=False, num_devices=NC,
                   num_swdge_queues=4, dynamic_dma_scratch_size=32768)

    x_in = nc.dram_tensor("x_in", [128, HNPC], fp32, kind="ExternalInput")
    w_in = nc.dram_tensor("w_in", [3 * 64, 64], fp32, kind="ExternalInput")
    bn_in = nc.dram_tensor("bn_in", [6 * 64, 1], fp32, kind="ExternalInput")
    deg_in = nc.dram_tensor("deg_in", [128, NT], fp32, kind="ExternalInput")
    idxg_in = nc.dram_tensor("idxg_in", [128, TOTC], i16, kind="ExternalInput")
    idxp_in = nc.dram_tensor("idxp_in", [128, pw_off[-1]], i16, kind="ExternalInput")
    hasw_in = nc.dram_tensor("hasw_in", [128, W], fp32, kind="ExternalInput")
    hofs_in = nc.dram_tensor("hofs_in", [128, W], fp32, kind="ExternalInput")
    npad_in = nc.dram_tensor("npad_in", [128, W], fp32, kind="ExternalInput")
    anym_in = nc.dram_tensor("anym_in", [128, 1], fp32, kind="ExternalInput")
    phm_in = nc.dram_tensor("phm_in", [128, 4], fp32, kind="ExternalInput")
    cntinv_in = nc.dram_tensor("cntinv_in", [64, GPC], fp32, kind="ExternalInput")
    out_ext = nc.dram_tensor("out", [G, D], fp32, kind="ExternalOutput")

    slice_q = [nc.dram_tensor(f"slice_q{w}", [QSZ[w], D], fp32) for w in range(W)]
    win_d = [nc.dram_tensor(f"win_d{w}", [WSZL[w], D], fp32, addr_space="Shared")
             for w in range(W)]
    stats_i = nc.dram_tensor("stats_i", [64, 2], fp32)
    stats_o = nc.dram_tensor("stats_o", [64, 2], fp32, addr_space="Shared")
    oslice_d = nc.dram_tensor("oslice_d", [GPC, D], fp32)
    ofull_d = nc.dram_tensor("ofull_d", [G, D], fp32, addr_space="Shared")

    RG = [list(range(NC))]
    INVN = 1.0 / float(N)
    TQ = [0, 25, 50, 75, 98]          # tile ranges per quarter

    with tile.TileContext(nc) as tc:
        with (
            tc.tile_pool(name="cp", bufs=1) as cp,
            tc.tile_pool(name="hp", bufs=2) as hp,
            tc.tile_pool(name="sm", bufs=4) as sm,
            tc.tile_pool(name="slb", bufs=4) as slb,
            tc.tile_pool(name="rp", bufs=3) as rp,
            tc.tile_pool(name="ixp", bufs=4) as ixp,
            tc.tile_pool(name="ps", bufs=2, space="PSUM") as ps,
        ):
            nc.gpsimd.load_library(mlp)
            ident = cp.tile([128, 128], fp32, tag="ident")
            make_identity(nc, ident[:])

            deg_sb = cp.tile([128, NT], fp32, tag="deg")
            nc.sync.dma_start(out=deg_sb[:], in_=deg_in[:])
            dinv = cp.tile([128, NT], fp32, tag="dinv")
            nc.vector.reciprocal(out=dinv[:], in_=deg_sb[:])
            nc.scalar.activation(out=dinv[:], in_=dinv[:],
                                 func=mybir.ActivationFunctionType.Sqrt)

            w_sb = []
            bn_sb = []

            def emit_body():
                h_cur = hp.tile([128, HNPC], fp32, tag="h")
                nc.sync.dma_start(out=h_cur[:], in_=x_in[:])
                for l in range(3):
                    wt = cp.tile([128, 64], fp32, tag=f"w{l}")
                    nc.sync.dma_start(out=wt[0:64, :], in_=w_in[l * 64:(l + 1) * 64, :])
                    nc.sync.dma_start(out=wt[64:128, :], in_=w_in[l * 64:(l + 1) * 64, :])
                    w_sb.append(wt)
                    bt = cp.tile([64, 2], fp32, tag=f"bn{l}")
                    nc.sync.dma_start(out=bt[:, 0:1], in_=bn_in[(2 * l) * 64:(2 * l + 1) * 64, :])
                    nc.sync.dma_start(out=bt[:, 1:2], in_=bn_in[(2 * l + 1) * 64:(2 * l + 2) * 64, :])
                    bn_sb.append(bt)

                hhat = cp.tile([128, NT * 64], fp32, tag="hhat")
                yst = cp.tile([128, HNPC], fp32, tag="yst")
                stacc = cp.tile([128, NT], fp32, tag="stacc")
                sqacc = cp.tile([128, NT], fp32, tag="sqacc")

                idxp_sb = cp.tile([128, pw_off[-1]], i16, tag="idxp")
                nc.sync.dma_start(out=idxp_sb[:], in_=idxp_in[:])
                hasw_sb = cp.tile([128, W], fp32, tag="hasw")
                nc.sync.dma_start(out=hasw_sb[:], in_=hasw_in[:])
                hofs_sb = cp.tile([128, W], fp32, tag="hofs")
                nc.sync.dma_start(out=hofs_sb[:], in_=hofs_in[:])
                npad_sb = cp.tile([128, W], fp32, tag="npad")
                nc.sync.dma_start(out=npad_sb[:], in_=npad_in[:])
                anym_sb = cp.tile([128, 1], fp32, tag="anym")
                nc.sync.dma_start(out=anym_sb[:], in_=anym_in[:])
                cinv_sb = cp.tile([64, GPC], fp32, tag="cinv")
                nc.sync.dma_start(out=cinv_sb[:], in_=cntinv_in[:])
                phm_sb = cp.tile([128, 4], fp32, tag="phm")
                nc.sync.dma_start(out=phm_sb[:], in_=phm_in[:])

                def emit_gemm_layer(l, src_h):
                    # h layout: rows 0:64 = features for nodes 0..6271 (cols),
                    # rows 64:128 = features for nodes 6272..12543.
                    for t in range(NT):
                        half, c = divmod(t, 49)
                        pb = 64 * half
                        cols = slice(c * 128, (c + 1) * 128)
                        pt = ps.tile([128, 64], fp32, tag="pt", space="PSUM")
                        nc.tensor.matmul(
                            out=pt[:], lhsT=src_h[pb:pb + 64, cols],
                            rhs=w_sb[l][pb:pb + 64, :], start=True, stop=True,
                        )
                        nc.scalar.activation(
                            out=hhat[:, t * 64:(t + 1) * 64], in_=pt[:],
                            func=mybir.ActivationFunctionType.Copy,
                            scale=dinv[:, t:t + 1])

                def stage_and_gather_tables(extra_mask=False):
                    if extra_mask:
                        for qi, t in enumerate(_PHM_TILES):
                            nc.vector.tensor_scalar_mul(
                                out=hhat[:, t * 64:(t + 1) * 64],
                                in0=hhat[:, t * 64:(t + 1) * 64],
                                scalar1=phm_sb[:, qi:qi + 1])
                    for w in range(W):
                        sl_v = slice_q[w][:].rearrange("(t p) d -> p t d", p=128)
                        nc.sync.dma_start(
                            out=sl_v,
                            in_=hhat[:, TQ[w] * 64:TQ[w + 1] * 64])
                        nc.gpsimd.collective_compute(
                            "AllGather", mybir.AluOpType.bypass, replica_groups=RG,
                            ins=[slice_q[w][:].opt()], outs=[win_d[w][:].opt()],
                        )

                for l in range(3):
                    emit_gemm_layer(l, h_cur)
                    stage_and_gather_tables()

                    # ---- batched gathers + reduce -> yst ----
                    for gg in ginfo:
                        t0, Gg, Kw, SKg = gg["t0"], gg["G"], gg["Kw"], gg["SKg"]
                        idxt = ixp.tile([128, GCOLS_MAX], i16, tag="idxg")
                        nc.sync.dma_start(
                            out=idxt[:, 0:gg["gcols"]],
                            in_=idxg_in[:, gg["goff"]:gg["goff"] + gg["gcols"]])
                        slab = slb.tile([128, CAP * 64], fp32, tag="slab")
                        for w in range(W):
                            if Kw[w] == 0:
                                continue
                            NIw = 128 * Gg * Kw[w]
                            ob = gg["wbase"][w] * 64
                            outv = slab[:, ob:ob + Gg * Kw[w] * 64].rearrange(
                                "p (j d) -> p j d", d=64)
                            nc.gpsimd.dma_gather(
                                outv,
                                win_d[w][:],
                                idxt[:, gg["woff"][w]:gg["woff"][w] + 8 * Gg * Kw[w]],
                                NIw, NIw, 64, single_packet=False, queue_num=w,
                            )
                        rws = rp.tile([128, W * Gg * 64], fp32, tag="rws")
                        nw = 0
                        for w in range(W):
                            if Kw[w] == 0:
                                continue
                            ob = gg["wbase"][w] * 64
                            dstv = rws[:, nw * Gg * 64:(nw + 1) * Gg * 64]
                            if Kw[w] == 1:
                                nc.vector.tensor_copy(out=dstv, in_=slab[:, ob:ob + Gg * 64])
                            else:
                                rin = slab[:, ob:ob + Gg * Kw[w] * 64].rearrange(
                                    "p (t k d) -> p t d k", t=Gg, k=Kw[w])
                                nc.vector.reduce_sum(out=dstv, in_=rin,
                                                     axis=mybir.AxisListType.X)
                            nw += 1
                        msum = rp.tile([128, Gg * 64], fp32, tag="msum")
                        if nw == 1:
                            nc.vector.tensor_tensor(
                                out=msum[:], in0=rws[:, 0:Gg * 64],
                                in1=hhat[:, t0 * 64:(t0 + Gg) * 64],
                                op=mybir.AluOpType.add)
                        else:
                            cin = rws[:, 0:nw * Gg * 64].rearrange(
                                "p (w x) -> p x w", w=nw)
                            nc.vector.reduce_sum(out=msum[:], in_=cin,
                                                 axis=mybir.AxisListType.X)
                            nc.vector.tensor_tensor(
                                out=msum[:], in0=msum[:],
                                in1=hhat[:, t0 * 64:(t0 + Gg) * 64],
                                op=mybir.AluOpType.add)
                        msum3 = msum[:, 0:Gg * 64].rearrange("p (t d) -> p t d", d=64)
                        db = dinv[:, t0:t0 + Gg][:, :, None].broadcast_to(
                            [128, Gg, 64])
                        nc.vector.tensor_tensor(out=msum3, in0=msum3, in1=db,
                                                op=mybir.AluOpType.mult)
                        for tloc in range(Gg):
                            t = t0 + tloc
                            half, c = divmod(t, 49)
                            pb = 64 * half
                            pyt = ps.tile([64, 128], fp32, tag="pyt", space="PSUM")
                            nc.tensor.transpose(
                                out=pyt[:], in_=msum[:, tloc * 64:(tloc + 1) * 64],
                                identity=ident[:, 0:128])
                            nc.scalar.activation(
                                out=yst[pb:pb + 64, c * 128:(c + 1) * 128], in_=pyt[:],
                                func=mybir.ActivationFunctionType.Copy)
                            # incremental BN stats (sum from PSUM, sq from SBUF)
                            nc.vector.reduce_sum(out=stacc[0:64, t:t + 1], in_=pyt[:],
                                                 axis=mybir.AxisListType.X)
                            ycols = yst[pb:pb + 64, c * 128:(c + 1) * 128]
                            sqt = sm.tile([64, 128], fp32, tag="sqt")
                            nc.vector.tensor_tensor(out=sqt[:], in0=ycols, in1=ycols,
                                                    op=mybir.AluOpType.mult)
                            nc.vector.reduce_sum(out=sqacc[0:64, t:t + 1], in_=sqt[:],
                                                 axis=mybir.AxisListType.X)

                    # ---- BN stats final combine ----
                    stt = sm.tile([128, 2], fp32, tag="stt")
                    nc.vector.reduce_sum(out=stt[0:64, 0:1], in_=stacc[0:64, :],
                                         axis=mybir.AxisListType.X)
                    nc.vector.reduce_sum(out=stt[0:64, 1:2], in_=sqacc[0:64, :],
                                         axis=mybir.AxisListType.X)
                    nc.sync.dma_start(out=stats_i[:], in_=stt[0:64, :])
                    nc.gpsimd.collective_compute(
                        "AllReduce", mybir.AluOpType.add, replica_groups=RG,
                        ins=[stats_i[:].opt()], outs=[stats_o[:].opt()],
                    )
                    if l == 2:
                        break   # BN3 folded into pooled output; stats AllReduce in flight

                    stin = sm.tile([64, 2], fp32, tag="stin")
                    nc.sync.dma_start(out=stin[:], in_=stats_o[:])

                    # ---- BN coefficients ----
                    co = sm.tile([64, 8], fp32, tag="co")
                    mean, ex2, m2, var, rec, rstd = (co[:, i:i + 1] for i in range(6))
                    nc.vector.tensor_scalar_mul(out=mean, in0=stin[:, 0:1], scalar1=INVN)
                    nc.vector.tensor_scalar_mul(out=ex2, in0=stin[:, 1:2], scalar1=INVN)
                    nc.vector.tensor_tensor(out=m2, in0=mean, in1=mean, op=mybir.AluOpType.mult)
                    nc.vector.tensor_tensor(out=var, in0=ex2, in1=m2, op=mybir.AluOpType.subtract)
                    nc.vector.tensor_scalar_add(out=var, in0=var, scalar1=float(EPS))
                    nc.vector.reciprocal(out=rec, in_=var)
                    nc.scalar.activation(out=rstd, in_=rec, func=mybir.ActivationFunctionType.Sqrt)
                    scsh = sm.tile([128, 2], fp32, tag="scsh")
                    nc.vector.tensor_tensor(out=scsh[0:64, 0:1], in0=bn_sb[l][:, 0:1],
                                            in1=rstd, op=mybir.AluOpType.mult)
                    ms = co[:, 6:7]
                    nc.vector.tensor_tensor(out=ms, in0=mean, in1=scsh[0:64, 0:1],
                                            op=mybir.AluOpType.mult)
                    nc.vector.tensor_tensor(out=scsh[0:64, 1:2], in0=bn_sb[l][:, 1:2],
                                            in1=ms, op=mybir.AluOpType.subtract)
                    nc.vector.tensor_copy(out=scsh[64:128, :], in_=scsh[0:64, :])

                    # ---- BN apply (+ReLU) -> next h ----
                    h_nxt = hp.tile([128, HNPC], fp32, tag="h")
                    for half in range(2):
                        pb = 64 * half
                        nc.scalar.activation(
                            out=h_nxt[pb:pb + 64, :], in_=yst[pb:pb + 64, :],
                            func=mybir.ActivationFunctionType.Relu,
                            bias=scsh[pb:pb + 64, 1:2], scale=scsh[pb:pb + 64, 0:1])
                    h_cur = h_nxt

                # ---- y3 (pre-BN) -> table (node-major transposes + zero masks) ----
                for t in range(NT):
                    half, c = divmod(t, 49)
                    pb = 64 * half
                    ph = ps.tile([128, 64], fp32, tag="ptr", space="PSUM")
                    nc.tensor.transpose(out=ph[:], in_=yst[pb:pb + 64, c * 128:(c + 1) * 128],
                                        identity=ident[pb:pb + 64, pb:pb + 64])
                    nc.scalar.activation(out=hhat[:, t * 64:(t + 1) * 64], in_=ph[:],
                                         func=mybir.ActivationFunctionType.Copy)
                stage_and_gather_tables(extra_mask=True)

                # ---- pooling (pre-BN): per-window batched gathers ----
                ssum = sm.tile([128, 64], fp32, tag="ssum")
                smax = sm.tile([128, 64], fp32, tag="smax")
                smin = sm.tile([128, 64], fp32, tag="smin")
                for w in range(W):
                    kw = KPW[w]
                    nip = 128 * kw
                    slabp = slb.tile([128, CAP * 64], fp32, tag="slab")
                    assert kw <= CAP
                    outv = slabp[:, 0:kw * 64].rearrange("p (j d) -> p j d", d=64)
                    nc.gpsimd.dma_gather(
                        outv, win_d[w][:],
                        idxp_sb[:, pw_off[w]:pw_off[w + 1]],
                        nip, nip, 64, single_packet=False, queue_num=w,
                    )
                    pv = slabp[:, 0:kw * 64].rearrange("p (k d) -> p d k", k=kw)
                    ts_ = sm.tile([128, 64], fp32, tag="tsum")
                    tm_ = sm.tile([128, 64], fp32, tag="tmax")
                    tn_ = sm.tile([128, 64], fp32, tag="tmin")
                    if kw == 1:
                        nc.vector.tensor_copy(out=ts_[:], in_=slabp[:, 0:64])
                        nc.vector.tensor_copy(out=tm_[:], in_=slabp[:, 0:64])
                        nc.vector.tensor_copy(out=tn_[:], in_=slabp[:, 0:64])
                    else:
                        nc.vector.reduce_sum(out=ts_[:], in_=pv,
                                             axis=mybir.AxisListType.X)
                        nc.vector.reduce_max(out=tm_[:], in_=pv,
                                             axis=mybir.AxisListType.X)
                        nc.vector.tensor_reduce(out=tn_[:], in_=pv,
                                                axis=mybir.AxisListType.X,
                                                op=mybir.AluOpType.min)
                    # sum correction: pads duplicated first member npad_w times
                    corr = sm.tile([128, 64], fp32, tag="corr")
                    nc.vector.tensor_scalar_mul(out=corr[:], in0=slabp[:, 0:64],
                                                scalar1=npad_sb[:, w:w + 1])
                    nc.vector.tensor_tensor(out=ts_[:], in0=ts_[:], in1=corr[:],
                                            op=mybir.AluOpType.subtract)
                    # mask missing windows: tm*has - 1e30*(1-has); tn*has + 1e30*(1-has)
                    nc.vector.tensor_scalar(
                        out=tm_[:], in0=tm_[:],
                        scalar1=hasw_sb[:, w:w + 1], scalar2=hofs_sb[:, w:w + 1],
                        op0=mybir.AluOpType.mult, op1=mybir.AluOpType.add)
                    nc.vector.tensor_scalar(
                        out=tn_[:], in0=tn_[:],
                        scalar1=hasw_sb[:, w:w + 1], scalar2=hofs_sb[:, w:w + 1],
                        op0=mybir.AluOpType.mult, op1=mybir.AluOpType.subtract)
                    if w == 0:
                        nc.vector.tensor_copy(out=ssum[:], in_=ts_[:])
                        nc.vector.tensor_copy(out=smax[:], in_=tm_[:])
                        nc.vector.tensor_copy(out=smin[:], in_=tn_[:])
                    else:
                        nc.vector.tensor_tensor(out=ssum[:], in0=ssum[:], in1=ts_[:],
                                                op=mybir.AluOpType.add)
                        nc.vector.tensor_tensor(out=smax[:], in0=smax[:], in1=tm_[:],
                                                op=mybir.AluOpType.max)
                        nc.vector.tensor_tensor(out=smin[:], in0=smin[:], in1=tn_[:],
                                                op=mybir.AluOpType.min)
                nc.vector.tensor_scalar_mul(out=smax[:], in0=smax[:],
                                            scalar1=anym_sb[:, 0:1])
                nc.vector.tensor_scalar_mul(out=smin[:], in0=smin[:],
                                            scalar1=anym_sb[:, 0:1])

                def to_fm(srct, tg):
                    p = ps.tile([64, 128], fp32, tag="pyt", space="PSUM")
                    nc.tensor.transpose(out=p[:], in_=srct[:], identity=ident[:, 0:128])
                    tt = sm.tile([64, 128], fp32, tag="fm" + tg)
                    nc.vector.tensor_copy(out=tt[:], in_=p[:])
                    return tt

                sfm = to_fm(ssum, "s")
                mfm = to_fm(smax, "m")
                nfm = to_fm(smin, "n")

                def qcombine(tsrc, op, tg):
                    v = tsrc[:].rearrange("f (g q) -> f q g", q=4)
                    a = sm.tile([64, GPC], fp32, tag="qa" + tg)
                    b = sm.tile([64, GPC], fp32, tag="qb" + tg)
                    nc.vector.tensor_tensor(out=a[:], in0=v[:, 0, :], in1=v[:, 1, :], op=op)
                    nc.vector.tensor_tensor(out=b[:], in0=v[:, 2, :], in1=v[:, 3, :], op=op)
                    nc.vector.tensor_tensor(out=a[:], in0=a[:], in1=b[:], op=op)
                    return a

                s32 = qcombine(sfm, mybir.AluOpType.add, "s")
                m32 = qcombine(mfm, mybir.AluOpType.max, "m")
                n32 = qcombine(nfm, mybir.AluOpType.min, "n")

                # ---- BN3 coefficients (AllReduce overlapped with pooling) ----
                stin = sm.tile([64, 2], fp32, tag="stin")
                nc.sync.dma_start(out=stin[:], in_=stats_o[:])
                co = sm.tile([64, 8], fp32, tag="co")
                mean, ex2, m2, var, rec, rstd = (co[:, i:i + 1] for i in range(6))
                nc.vector.tensor_scalar_mul(out=mean, in0=stin[:, 0:1], scalar1=INVN)
                nc.vector.tensor_scalar_mul(out=ex2, in0=stin[:, 1:2], scalar1=INVN)
                nc.vector.tensor_tensor(out=m2, in0=mean, in1=mean, op=mybir.AluOpType.mult)
                nc.vector.tensor_tensor(out=var, in0=ex2, in1=m2, op=mybir.AluOpType.subtract)
                nc.vector.tensor_scalar_add(out=var, in0=var, scalar1=float(EPS))
                nc.vector.reciprocal(out=rec, in_=var)
                nc.scalar.activation(out=rstd, in_=rec, func=mybir.ActivationFunctionType.Sqrt)
                scsh = sm.tile([128, 2], fp32, tag="scsh")
                nc.vector.tensor_tensor(out=scsh[0:64, 0:1], in0=bn_sb[2][:, 0:1],
                                        in1=rstd, op=mybir.AluOpType.mult)
                ms = co[:, 6:7]
                nc.vector.tensor_tensor(out=ms, in0=mean, in1=scsh[0:64, 0:1],
                                        op=mybir.AluOpType.mult)
                nc.vector.tensor_tensor(out=scsh[0:64, 1:2], in0=bn_sb[2][:, 1:2],
                                        in1=ms, op=mybir.AluOpType.subtract)

                # out = BN(mean) + max(BN(max), BN(min)) per feature
                outfm = sm.tile([64, GPC], fp32, tag="outfm")
                nc.vector.tensor_tensor(out=outfm[:], in0=s32[:], in1=cinv_sb[:],
                                        op=mybir.AluOpType.mult)
                nc.vector.tensor_scalar(
                    out=outfm[:], in0=outfm[:], scalar1=scsh[0:64, 0:1],
                    scalar2=scsh[0:64, 1:2],
                    op0=mybir.AluOpType.mult, op1=mybir.AluOpType.add)
                nc.vector.tensor_scalar(
                    out=m32[:], in0=m32[:], scalar1=scsh[0:64, 0:1],
                    scalar2=scsh[0:64, 1:2],
                    op0=mybir.AluOpType.mult, op1=mybir.AluOpType.add)
                nc.vector.tensor_scalar(
                    out=n32[:], in0=n32[:], scalar1=scsh[0:64, 0:1],
                    scalar2=scsh[0:64, 1:2],
                    op0=mybir.AluOpType.mult, op1=mybir.AluOpType.add)
                nc.vector.tensor_tensor(out=m32[:], in0=m32[:], in1=n32[:],
                                        op=mybir.AluOpType.max)
                nc.vector.tensor_tensor(out=outfm[:], in0=outfm[:], in1=m32[:],
                                        op=mybir.AluOpType.add)
                po = ps.tile([GPC, 64], fp32, tag="ptr", space="PSUM")
                nc.tensor.transpose(out=po[:], in_=outfm[:], identity=ident[0:64, 0:64])
                onm = sm.tile([GPC, 64], fp32, tag="onm")
                nc.vector.tensor_copy(out=onm[:], in_=po[:])
                nc.sync.dma_start(out=oslice_d[:], in_=onm[:])
                nc.gpsimd.collective_compute(
                    "AllGather", mybir.AluOpType.bypass, replica_groups=RG,
                    ins=[oslice_d[:].opt()], outs=[ofull_d[:].opt()],
                )
                for half in range(2):
                    ot = sm.tile([128, 64], fp32, tag="ot")
                    nc.sync.dma_start(out=ot[:], in_=ofull_d[half * 128:(half + 1) * 128, :])
                    nc.sync.dma_start(out=out_ext[half * 128:(half + 1) * 128, :], in_=ot[:])

            emit_body()

    nc.compile()
    return nc


def _make_inmaps(x, prep, Ws, gs, bes):
    lpos = prep["lpos"]
    core = np.arange(N) // NPC_RAW
    w_np = np.concatenate(Ws, axis=0).astype(np.float32)
    bn_np = np.zeros((6 * 64, 1), dtype=np.float32)
    for l in range(3):
        bn_np[(2 * l) * 64:(2 * l + 1) * 64, 0] = gs[l]
        bn_np[(2 * l + 1) * 64:(2 * l + 2) * 64, 0] = bes[l]
    xl = np.zeros((NC, NPC, D), dtype=np.float32)
    xl[core, lpos] = x
    phm = _zero_masks()
    in_maps = []
    for c in range(NC):
        sl = xl[c]
        xs = np.zeros((128, HNPC), dtype=np.float32)
        xs[0:64, :] = sl[:HNPC].T
        xs[64:128, :] = sl[HNPC:].T
        in_maps.append({
            "x_in": xs,
            "w_in": w_np,
            "bn_in": bn_np,
            "deg_in": prep["deg_f"][c],
            "idxg_in": prep["idxg"][c],
            "idxp_in": prep["idxPg"][c],
            "hasw_in": prep["hasw"][c],
            "hofs_in": prep["hofs"][c],
            "npad_in": prep["npadw"][c],
            "anym_in": prep["any_m"][c][:, None].copy(),
            "phm_in": phm,
            "cntinv_in": prep["cntinv_fm"][c],
        })
    return in_maps


def _run_device(x, prep, Ws, gs, bes):
    from concourse.bass_utils import run_bass_kernel_spmd

    import os
    key = (prep["TOTC"], tuple(prep["KPW"]),
           tuple(tuple(g["Kw"]) for g in prep["ginfo"]))
    if key not in _DEVICE_CACHE:
        _DEVICE_CACHE[key] = _build_device(prep["ginfo"], prep["TOTC"],
                                           prep["KPW"], prep["pw_off"])
    nc = _DEVICE_CACHE[key]
    in_maps = _make_inmaps(x, prep, Ws, gs, bes)
    trace = bool(os.environ.get("GCN_TRACE"))
    kw = {}
    if trace:
        kw["trace"] = True
        td = os.environ.get("GCN_TRACE_DIR")
        if td:
            os.makedirs(td, exist_ok=True)
            kw["tmpdir"] = td
    res = run_bass_kernel_spmd(nc, in_maps, core_ids=list(range(NC)), **kw)
    global _LAST_RES
    _LAST_RES = res
    return np.asarray(res.results[0]["out"], dtype=np.float32)
